# revision 1
# baseline (speedup 1.0000x reference)
"""Trainium2 Bass kernel for nn_CausalFlowModel.

Model: encoder MLP -> discretised-LSTM scan over T=1024 -> interpolated
select at per-sample index -> decoder MLP.

Key algebraic trick: the reference computes
    enc[b,t] = (1-d[b,t]) * h[b,t-1] + d[b,t] * h[b,t]
and selects enc[b, idx_b].  Since h[b,t] = h[b,t-1] + d*(h_cand - h[b,t-1]),
enc[b, idx_b] = h[b,idx-1] + d^2*(h_cand - h[b,idx-1]).  So feeding the scan
modified deltas (d for t<idx, d^2 at t==idx, 0 after) makes the final h carry
equal the selected/interpolated value -- no [B,T,Z] materialisation, roll or
gather.

Sharding: data-parallel, batch 512 -> 8 cores x 64.

Per-core layout: batch-major state X[64, 144] = [c|h]; h_fm[72,64]
(feature-major h, regenerated each step by PE transpose + copy) is the lhsT
for the recurrent matmul.  gates[64,288] = u_t_aug @ Wih_aug + h @ Whh via
2 PE matmuls into PSUM.  Gate columns permuted to [i|f|o|g] so one sigmoid
covers 216 cols; z-dim permuted to [h0|x] so partition slices stay aligned.

All constants/weights are packed into a single [128, PACK_COLS] DRAM tensor
(one DMA) to keep per-instruction sync-wait counts low.
"""

import numpy as np

import concourse.bass as bass
import concourse.bacc as bacc
import concourse.tile as tile
from concourse import mybir
from concourse.bass_utils import run_bass_kernel_spmd

B, T = 512, 1024
SD, CD = 8, 4
CRS = 64
Z = CRS + SD            # 72
G4 = 4 * Z              # 288
ENC_H = 128
DEC_H = 2 * Z           # 144
OUT = 8
NCORES = 8
BC = B // NCORES        # 64 batch per core

FP = mybir.dt.float32
U_CHUNK = 128           # time steps per u-DMA chunk

# packed-constants column layout: name -> (rows, col_off, cols)
_PACK = {}
_pc = 0
for _name, _r, _c in [
    ("wih", 6, G4), ("whh", Z, G4),
    ("we1", SD, ENC_H), ("we2", ENC_H, ENC_H), ("we3", ENC_H, CRS),
    ("be1", ENC_H, 1), ("be2", ENC_H, 1), ("be3", CRS, 1),
    ("wd1", Z, DEC_H), ("wd2a", 128, DEC_H), ("wd2b", 16, DEC_H),
    ("wd3a", 128, OUT), ("wd3b", 16, OUT),
    ("bd1", 128, 1), ("bd1b", 16, 1), ("bd2", 128, 1), ("bd2b", 16, 1),
    ("bd3", OUT, 1),
    ("ident", Z, Z), ("xfm", SD, BC), ("dmod", BC, T),
]:
    _PACK[_name] = (_r, _pc, _c)
    _pc += _c
PACK_COLS = _pc


def _build_bass():
    nc = bacc.Bacc("TRN2", target_bir_lowering=False, debug=False)

    pack_d = nc.declare_dram_parameter("pack", [128, PACK_COLS], FP,
                                       isOutput=False)
    u_d = nc.declare_dram_parameter("u", [6, T * BC], FP, isOutput=False)
    y_d = nc.declare_dram_parameter("y", [OUT, BC], FP, isOutput=True)

    with tile.TileContext(nc) as tc:
        with (
            tc.tile_pool(name="w", bufs=1) as wp,
            tc.tile_pool(name="state", bufs=1) as sp,
            tc.tile_pool(name="u", bufs=2) as up,
            tc.tile_pool(name="work", bufs=2) as kp,
            tc.tile_pool(name="ps", bufs=2, space="PSUM") as pp,
            tc.tile_pool(name="pst", bufs=2, space="PSUM") as pt,
        ):
            pack = wp.tile([128, PACK_COLS], FP, name="pack_sb", tag="pack_sb")
            nc.gpsimd.dma_start(pack[:], pack_d[:])

            def pk(name):
                r, c0, c = _PACK[name]
                return pack[0:r, c0:c0 + c]

            wih, whh = pk("wih"), pk("whh")
            we1, we2, we3 = pk("we1"), pk("we2"), pk("we3")
            be1, be2, be3 = pk("be1"), pk("be2"), pk("be3")
            wd1, wd2a, wd2b = pk("wd1"), pk("wd2a"), pk("wd2b")
            wd3a, wd3b = pk("wd3a"), pk("wd3b")
            bd1, bd1b, bd2, bd2b, bd3 = (pk("bd1"), pk("bd1b"), pk("bd2"),
                                         pk("bd2b"), pk("bd3"))
            ident, xfm, dmod = pk("ident"), pk("xfm"), pk("dmod")

            # ---- persistent state ----
            h_fm = sp.tile([Z, BC], FP)       # feature-major h (lhsT for Whh mm)
            X = sp.tile([BC, 2 * Z], FP)      # [c | h] batch-major

            # ---- encoder MLP (feature-major) -> z0 ----
            TANH = mybir.ActivationFunctionType.Tanh
            SIG = mybir.ActivationFunctionType.Sigmoid

            ep1 = pp.tile([ENC_H, BC], FP, tag="mlp")
            nc.tensor.matmul(ep1[:], we1, xfm, start=True, stop=True)
            e1 = kp.tile([ENC_H, BC], FP, tag="enc")
            nc.scalar.activation(e1[:], ep1[:], TANH, bias=be1)
            ep2 = pp.tile([ENC_H, BC], FP, tag="mlp")
            nc.tensor.matmul(ep2[:], we2, e1[:], start=True, stop=True)
            e2 = kp.tile([ENC_H, BC], FP, tag="enc")
            nc.scalar.activation(e2[:], ep2[:], TANH, bias=be2)
            ep3 = pp.tile([CRS, BC], FP, tag="mlp")
            nc.tensor.matmul(ep3[:], we3, e2[:], start=True, stop=True)
            # z0_fm, permuted layout [h0 | x]: rows 0:64 = h0, rows 64:72 = x^T
            nc.vector.tensor_scalar_add(h_fm[0:CRS, :], ep3[:], be3)
            nc.vector.tensor_copy(h_fm[CRS:Z, :], xfm)

            # batch-major init: c0 = 0, h0 = transpose(z0_fm)
            nc.vector.memset(X[:, 0:Z], 0.0)
            zt_ps = pt.tile([BC, Z], FP, tag="hT")
            nc.tensor.transpose(zt_ps[:], h_fm[:], ident)
            nc.vector.tensor_copy(X[:, Z:2 * Z], zt_ps[:])

            # ---- the scan ----
            n_chunks = T // U_CHUNK
            for ci in range(n_chunks):
                u_sb = up.tile([6, U_CHUNK * BC], FP, tag="u")
                nc.gpsimd.dma_start(
                    u_sb[:],
                    u_d[:, ci * U_CHUNK * BC:(ci + 1) * U_CHUNK * BC])
                for tl in range(U_CHUNK):
                    t = ci * U_CHUNK + tl
                    u_t = u_sb[:, tl * BC:(tl + 1) * BC]
                    d_t = dmod[:, t:t + 1]

                    gp = pp.tile([BC, G4], FP, tag="gates")
                    nc.tensor.matmul(gp[:], u_t, wih, start=True, stop=False)
                    nc.tensor.matmul(gp[:], h_fm[:], whh, start=False, stop=True)

                    S = kp.tile([BC, 3 * Z], FP, tag="S")      # sig(i,f,o)
                    nc.scalar.activation(S[:], gp[:, 0:3 * Z], SIG)
                    gt = kp.tile([BC, Z], FP, tag="gt")        # tanh(g)
                    nc.scalar.activation(gt[:], gp[:, 3 * Z:G4], TANH)

                    ig = kp.tile([BC, Z], FP, tag="ig")
                    nc.vector.tensor_mul(ig[:], S[:, 0:Z], gt[:])
                    Xc = kp.tile([BC, 2 * Z], FP, tag="Xc")    # [c_cand|h_cand]
                    nc.vector.tensor_mul(Xc[:, 0:Z], S[:, Z:2 * Z], X[:, 0:Z])
                    nc.vector.tensor_add(Xc[:, 0:Z], Xc[:, 0:Z], ig[:])
                    th = kp.tile([BC, Z], FP, tag="th")
                    nc.scalar.activation(th[:], Xc[:, 0:Z], TANH)
                    nc.vector.tensor_mul(Xc[:, Z:2 * Z], S[:, 2 * Z:3 * Z], th[:])

                    dX = kp.tile([BC, 2 * Z], FP, tag="dX")
                    nc.vector.tensor_sub(dX[:], Xc[:], X[:])
                    nc.vector.tensor_scalar_mul(dX[:], dX[:], d_t)
                    nc.vector.tensor_add(X[:], X[:], dX[:])

                    hT = pt.tile([Z, BC], FP, tag="hT")
                    nc.tensor.transpose(hT[:], X[:, Z:2 * Z], ident[0:BC, 0:BC])
                    nc.vector.tensor_copy(h_fm[:], hT[:])

            # ---- decoder MLP on sel = final h (feature-major h_fm) ----
            dp1 = pp.tile([128, BC], FP, tag="mlp")
            nc.tensor.matmul(dp1[:], wd1[:, 0:128], h_fm[:], start=True, stop=True)
            dp1b = pp.tile([16, BC], FP, tag="mlpb")
            nc.tensor.matmul(dp1b[:], wd1[:, 128:DEC_H], h_fm[:], start=True,
                             stop=True)
            d1 = kp.tile([128, BC], FP, tag="dec")
            nc.scalar.activation(d1[:], dp1[:], TANH, bias=bd1)
            d1b = kp.tile([16, BC], FP, tag="decb")
            nc.scalar.activation(d1b[:], dp1b[:], TANH, bias=bd1b)

            dp2 = pp.tile([128, BC], FP, tag="mlp")
            nc.tensor.matmul(dp2[:], wd2a[:, 0:128], d1[:], start=True, stop=False)
            nc.tensor.matmul(dp2[:], wd2b[:, 0:128], d1b[:], start=False, stop=True)
            dp2b = pp.tile([16, BC], FP, tag="mlpb")
            nc.tensor.matmul(dp2b[:], wd2a[:, 128:DEC_H], d1[:], start=True,
                             stop=False)
            nc.tensor.matmul(dp2b[:], wd2b[:, 128:DEC_H], d1b[:], start=False,
                             stop=True)
            d2 = kp.tile([128, BC], FP, tag="dec")
            nc.scalar.activation(d2[:], dp2[:], TANH, bias=bd2)
            d2b = kp.tile([16, BC], FP, tag="decb")
            nc.scalar.activation(d2b[:], dp2b[:], TANH, bias=bd2b)

            dp3 = pp.tile([OUT, BC], FP, tag="mlpb")
            nc.tensor.matmul(dp3[:], wd3a, d2[:], start=True, stop=False)
            nc.tensor.matmul(dp3[:], wd3b, d2b[:], start=False, stop=True)
            y = kp.tile([OUT, BC], FP, tag="y")
            nc.vector.tensor_scalar_add(y[:], dp3[:], bd3)
            nc.sync.dma_start(y_d[:], y[:])

    nc.compile()
    return nc


_NC_CACHE = None


def _get_nc():
    global _NC_CACHE
    if _NC_CACHE is None:
        _NC_CACHE = _build_bass()
    return _NC_CACHE


def _prep_core_inputs(inputs):
    """Host-side sharding + layout prep. Returns list of per-core in_maps."""
    x = np.asarray(inputs["x"], np.float32)
    rnn = np.asarray(inputs["rnn_input"], np.float32)
    deltas = np.asarray(inputs["deltas"], np.float32)[..., 0]     # [B,T]
    lengths = np.asarray(inputs["lengths"], np.int64)
    idx = np.clip(lengths - 1, 0, T - 1)                          # [B]

    # modified deltas: d for t<idx, d^2 at t==idx, 0 after
    tt = np.arange(T)[None, :]
    dmod = np.where(tt < idx[:, None], deltas,
                    np.where(tt == idx[:, None], deltas * deltas, 0.0)
                    ).astype(np.float32)

    # gate order [i|f|o|g]; within each gate, z order permuted to [h0|x]
    b = (np.asarray(inputs["bih"], np.float32)
         + np.asarray(inputs["bhh"], np.float32))
    perm_z = np.concatenate([np.arange(SD, Z), np.arange(0, SD)])
    gate_perm = np.concatenate([np.arange(0, Z), np.arange(Z, 2 * Z),
                                np.arange(3 * Z, 4 * Z), np.arange(2 * Z, 3 * Z)])
    col_perm = np.concatenate([gate_perm[blk * Z + perm_z] for blk in range(4)])
    wih = np.asarray(inputs["Wih"], np.float32)[:, col_perm]
    whh = np.asarray(inputs["Whh"], np.float32)[np.ix_(perm_z, col_perm)]
    bih_aug = b[col_perm][None, :]                                # [1, 288]
    wih_aug = np.concatenate([wih, bih_aug], axis=0)              # [6, 288]

    ones = np.ones((B, T, 1), np.float32)
    u_aug = np.concatenate([rnn, ones], axis=2)                   # [B, T, 6]

    wd2 = np.asarray(inputs["Wd2"], np.float32)
    wd3 = np.asarray(inputs["Wd3"], np.float32)
    wd1p = np.asarray(inputs["Wd1"], np.float32)[perm_z]
    consts = {
        "wih": wih_aug,
        "whh": whh,
        "we1": np.asarray(inputs["We1"], np.float32),
        "we2": np.asarray(inputs["We2"], np.float32),
        "we3": np.asarray(inputs["We3"], np.float32),
        "be1": np.asarray(inputs["be1"], np.float32).reshape(ENC_H, 1),
        "be2": np.asarray(inputs["be2"], np.float32).reshape(ENC_H, 1),
        "be3": np.asarray(inputs["be3"], np.float32).reshape(CRS, 1),
        "wd1": wd1p,
        "wd2a": wd2[0:128],
        "wd2b": wd2[128:DEC_H],
        "wd3a": wd3[0:128],
        "wd3b": wd3[128:DEC_H],
        "bd1": np.asarray(inputs["bd1"], np.float32)[0:128].reshape(128, 1),
        "bd1b": np.asarray(inputs["bd1"], np.float32)[128:].reshape(16, 1),
        "bd2": np.asarray(inputs["bd2"], np.float32)[0:128].reshape(128, 1),
        "bd2b": np.asarray(inputs["bd2"], np.float32)[128:].reshape(16, 1),
        "bd3": np.asarray(inputs["bd3"], np.float32).reshape(OUT, 1),
        "ident": np.eye(Z, dtype=np.float32),
    }

    base_pack = np.zeros((128, PACK_COLS), np.float32)
    for name, arr in consts.items():
        r, c0, c = _PACK[name]
        assert arr.shape == (r, c), (name, arr.shape, (r, c))
        base_pack[0:r, c0:c0 + c] = arr

    in_maps = []
    for k in range(NCORES):
        rows = slice(k * BC, (k + 1) * BC)
        p = base_pack.copy()
        r, c0, c = _PACK["xfm"]
        p[0:r, c0:c0 + c] = x[rows].T
        r, c0, c = _PACK["dmod"]
        p[0:r, c0:c0 + c] = dmod[rows]
        m = {
            "pack": p,
            "u": np.ascontiguousarray(u_aug[rows].transpose(2, 1, 0)
                                      ).reshape(6, T * BC),
        }
        in_maps.append(m)
    return in_maps


def kernel(**inputs):
    nc = _get_nc()
    in_maps = _prep_core_inputs(inputs)
    res = run_bass_kernel_spmd(nc, in_maps, core_ids=list(range(NCORES)))
    outs = [res.results[k]["y"].T for k in range(NCORES)]   # each [BC, OUT]
    return np.ascontiguousarray(np.concatenate(outs, axis=0).astype(np.float32))



# revision 2
# speedup vs baseline: 1.2453x; 1.2453x over previous
"""Trainium2 Bass kernel for nn_CausalFlowModel.

Model: encoder MLP -> discretised-LSTM scan over T=1024 -> interpolated
select at per-sample index -> decoder MLP.

Key algebraic trick (from baseline): the reference computes
    enc[b,t] = (1-d[b,t]) * h[b,t-1] + d[b,t] * h[b,t]
and selects enc[b, idx_b].  Since h[b,t] = h[b,t-1] + d*(h_cand - h[b,t-1]),
enc[b, idx_b] = h[b,idx-1] + d^2*(h_cand - h[b,idx-1]).  So feeding the scan
modified deltas (d for t<idx, d^2 at t==idx, 0 after) makes the final h carry
equal the selected/interpolated value -- no [B,T,Z] materialisation or gather.

v2 optimisations over baseline:
  * all matmuls in bf16 (fp32 matmul lowers to 2 passes ~1.2us; bf16 is one
    ~125ns pass).  State stays fp32; only matmul operands are rounded.
    Validated in numpy: rel err 4.3e-3 vs 2e-2 tolerance.
  * fused elementwise: tanh(g) is written adjacent to persistent [c|h] state
    so ONE tensor_tensor computes [i*gt | f*c]; the discretised update
    X += d*(Xc-X) is one tensor_sub + one scalar_tensor_tensor
    (out = (in0*scalar) + in1) with per-partition scalar d_t.
  * activations read PSUM directly (172- vs 222-cycle fixed cost).

Per-step chain (12 instrs): mm_u, mm_h -> tanh(g), sig(i,f,o) -> TT pair-mul,
TT add (c') -> tanh(c') -> TT (hc) -> TT sub, STT update -> PE transpose ->
copy/cast h to bf16 feature-major for the next step's matmul.

Sharding: data-parallel, batch 512 -> 8 cores x 64.
"""

import numpy as np
import ml_dtypes

import concourse.bass as bass
import concourse.bacc as bacc
import concourse.tile as tile
from concourse import mybir
from concourse.bass_utils import run_bass_kernel_spmd

B, T = 512, 1024
SD, CD = 8, 4
CRS = 64
Z = CRS + SD            # 72
G4 = 4 * Z              # 288
ENC_H = 128
DEC_H = 2 * Z           # 144
OUT = 8
NCORES = 8
BC = B // NCORES        # 64 batch per core

FP = mybir.dt.float32
BF = mybir.dt.bfloat16
U_CHUNK = 128           # time steps per u-DMA chunk

# fp32 packed-constants column layout: name -> (rows, col_off, cols)
_PACKF = {}
_pcf = 0
for _name, _r, _c in [
    ("we1", SD, ENC_H), ("we2", ENC_H, ENC_H), ("we3", ENC_H, CRS),
    ("be1", ENC_H, 1), ("be2", ENC_H, 1), ("be3", CRS, 1),
    ("bd1", 128, 1), ("bd1b", 16, 1), ("bd2", 128, 1), ("bd2b", 16, 1),
    ("bd3", OUT, 1),
    ("ident", Z, Z), ("xfm", SD, BC), ("dmod", BC, T),
]:
    _PACKF[_name] = (_r, _pcf, _c)
    _pcf += _c
PACKF_COLS = _pcf

# bf16 packed-constants column layout
_PACKB = {}
_pcb = 0
for _name, _r, _c in [
    ("wih", 6, G4), ("whh", Z, G4),
    ("wd1", Z, DEC_H), ("wd2a", 128, DEC_H), ("wd2b", 16, DEC_H),
    ("wd3a", 128, OUT), ("wd3b", 16, OUT),
]:
    _PACKB[_name] = (_r, _pcb, _c)
    _pcb += _c
PACKB_COLS = _pcb


def _build_bass():
    nc = bacc.Bacc("TRN2", target_bir_lowering=False, debug=False)

    packf_d = nc.declare_dram_parameter("packf", [128, PACKF_COLS], FP,
                                        isOutput=False)
    packb_d = nc.declare_dram_parameter("packb", [128, PACKB_COLS], BF,
                                        isOutput=False)
    u_d = nc.declare_dram_parameter("u", [6, T * BC], BF, isOutput=False)
    y_d = nc.declare_dram_parameter("y", [OUT, BC], FP, isOutput=True)

    TANH = mybir.ActivationFunctionType.Tanh
    SIG = mybir.ActivationFunctionType.Sigmoid
    MULT = mybir.AluOpType.mult
    ADD = mybir.AluOpType.add

    with tile.TileContext(nc) as tc:
        with (
            tc.tile_pool(name="w", bufs=1) as wp,
            tc.tile_pool(name="state", bufs=1) as sp,
            tc.tile_pool(name="u", bufs=2) as up,
            tc.tile_pool(name="work", bufs=2) as kp,
            tc.tile_pool(name="ps", bufs=2, space="PSUM") as pp,
            tc.tile_pool(name="pst", bufs=2, space="PSUM") as pt,
        ):
            packf = wp.tile([128, PACKF_COLS], FP, name="packf_sb",
                            tag="packf_sb")
            nc.gpsimd.dma_start(packf[:], packf_d[:])
            packb = wp.tile([128, PACKB_COLS], BF, name="packb_sb",
                            tag="packb_sb")
            nc.gpsimd.dma_start(packb[:], packb_d[:])

            def pkf(name):
                r, c0, c = _PACKF[name]
                return packf[0:r, c0:c0 + c]

            def pkb(name):
                r, c0, c = _PACKB[name]
                return packb[0:r, c0:c0 + c]

            wih, whh = pkb("wih"), pkb("whh")
            wd1, wd2a, wd2b = pkb("wd1"), pkb("wd2a"), pkb("wd2b")
            wd3a, wd3b = pkb("wd3a"), pkb("wd3b")
            we1, we2, we3 = pkf("we1"), pkf("we2"), pkf("we3")
            be1, be2, be3 = pkf("be1"), pkf("be2"), pkf("be3")
            bd1, bd1b, bd2, bd2b, bd3 = (pkf("bd1"), pkf("bd1b"), pkf("bd2"),
                                         pkf("bd2b"), pkf("bd3"))
            ident, xfm, dmod = pkf("ident"), pkf("xfm"), pkf("dmod")

            # ---- persistent state ----
            # W2 = [gt | c | h] batch-major; gt rewritten each step so the
            # pair-product [i*gt | f*c] is a single tensor_tensor.
            W2 = sp.tile([BC, 3 * Z], FP)
            S = sp.tile([BC, 3 * Z], FP)      # sigmoid(i,f,o)
            h_fmB = sp.tile([Z, BC], BF)      # feature-major h (bf16, lhsT)

            # ---- encoder MLP (feature-major, fp32) -> z0 ----
            h_fm32 = sp.tile([Z, BC], FP)
            ep1 = pp.tile([ENC_H, BC], FP, tag="mlp")
            nc.tensor.matmul(ep1[:], we1, xfm, start=True, stop=True)
            e1 = kp.tile([ENC_H, BC], FP, tag="enc")
            nc.scalar.activation(e1[:], ep1[:], TANH, bias=be1)
            ep2 = pp.tile([ENC_H, BC], FP, tag="mlp")
            nc.tensor.matmul(ep2[:], we2, e1[:], start=True, stop=True)
            e2 = kp.tile([ENC_H, BC], FP, tag="enc")
            nc.scalar.activation(e2[:], ep2[:], TANH, bias=be2)
            ep3 = pp.tile([CRS, BC], FP, tag="mlp")
            nc.tensor.matmul(ep3[:], we3, e2[:], start=True, stop=True)
            # z0_fm, permuted layout [h0 | x]: rows 0:64 = h0, rows 64:72 = x^T
            nc.vector.tensor_scalar_add(h_fm32[0:CRS, :], ep3[:], be3)
            nc.vector.tensor_copy(h_fm32[CRS:Z, :], xfm)
            nc.vector.tensor_copy(h_fmB[:], h_fm32[:])

            # batch-major init: c0 = 0, h0 = transpose(z0_fm)
            nc.vector.memset(W2[:, Z:2 * Z], 0.0)
            zt_ps = pt.tile([BC, Z], FP, tag="hT")
            nc.tensor.transpose(zt_ps[:], h_fm32[:], ident)
            nc.vector.tensor_copy(W2[:, 2 * Z:3 * Z], zt_ps[:])

            # ---- the scan ----
            n_chunks = T // U_CHUNK
            for ci in range(n_chunks):
                u_sb = up.tile([6, U_CHUNK * BC], BF, tag="u")
                nc.gpsimd.dma_start(
                    u_sb[:],
                    u_d[:, ci * U_CHUNK * BC:(ci + 1) * U_CHUNK * BC])
                for tl in range(U_CHUNK):
                    t = ci * U_CHUNK + tl
                    u_t = u_sb[:, tl * BC:(tl + 1) * BC]
                    d_t = dmod[:, t:t + 1]

                    gp = pp.tile([BC, G4], FP, tag="gates")
                    nc.tensor.matmul(gp[:], u_t, wih, start=True, stop=False)
                    nc.tensor.matmul(gp[:], h_fmB[:], whh, start=False,
                                     stop=True)

                    # gt into W2[:,0:72]; sig(i,f,o) into S
                    nc.scalar.activation(W2[:, 0:Z], gp[:, 3 * Z:G4], TANH)
                    nc.scalar.activation(S[:, 0:3 * Z], gp[:, 0:3 * Z], SIG)

                    P = kp.tile([BC, 2 * Z], FP, tag="P")
                    nc.vector.tensor_mul(P[:], S[:, 0:2 * Z], W2[:, 0:2 * Z])
                    Xc = kp.tile([BC, 2 * Z], FP, tag="Xc")
                    nc.vector.tensor_add(Xc[:, 0:Z], P[:, 0:Z], P[:, Z:2 * Z])
                    th = kp.tile([BC, Z], FP, tag="th")
                    nc.scalar.activation(th[:], Xc[:, 0:Z], TANH)
                    nc.vector.tensor_mul(Xc[:, Z:2 * Z], S[:, 2 * Z:3 * Z],
                                         th[:])

                    dX = kp.tile([BC, 2 * Z], FP, tag="dX")
                    nc.vector.tensor_sub(dX[:], Xc[:], W2[:, Z:3 * Z])
                    nc.vector.scalar_tensor_tensor(
                        W2[:, Z:3 * Z], dX[:], d_t, W2[:, Z:3 * Z], MULT, ADD)

                    hT = pt.tile([Z, BC], FP, tag="hT")
                    nc.tensor.transpose(hT[:], W2[:, 2 * Z:3 * Z],
                                        ident[0:BC, 0:BC])
                    nc.vector.tensor_copy(h_fmB[:], hT[:])

            # ---- decoder MLP on sel = final h (feature-major h_fmB) ----
            dp1 = pp.tile([128, BC], FP, tag="mlp")
            nc.tensor.matmul(dp1[:], wd1[:, 0:128], h_fmB[:], start=True,
                             stop=True)
            dp1b = pp.tile([16, BC], FP, tag="mlpb")
            nc.tensor.matmul(dp1b[:], wd1[:, 128:DEC_H], h_fmB[:], start=True,
                             stop=True)
            d1 = kp.tile([128, BC], BF, tag="dec")
            nc.scalar.activation(d1[:], dp1[:], TANH, bias=bd1)
            d1b = kp.tile([16, BC], BF, tag="decb")
            nc.scalar.activation(d1b[:], dp1b[:], TANH, bias=bd1b)

            dp2 = pp.tile([128, BC], FP, tag="mlp")
            nc.tensor.matmul(dp2[:], wd2a[:, 0:128], d1[:], start=True,
                             stop=False)
            nc.tensor.matmul(dp2[:], wd2b[:, 0:128], d1b[:], start=False,
                             stop=True)
            dp2b = pp.tile([16, BC], FP, tag="mlpb")
            nc.tensor.matmul(dp2b[:], wd2a[:, 128:DEC_H], d1[:], start=True,
                             stop=False)
            nc.tensor.matmul(dp2b[:], wd2b[:, 128:DEC_H], d1b[:], start=False,
                             stop=True)
            d2 = kp.tile([128, BC], BF, tag="dec")
            nc.scalar.activation(d2[:], dp2[:], TANH, bias=bd2)
            d2b = kp.tile([16, BC], BF, tag="decb")
            nc.scalar.activation(d2b[:], dp2b[:], TANH, bias=bd2b)

            dp3 = pp.tile([OUT, BC], FP, tag="mlpb")
            nc.tensor.matmul(dp3[:], wd3a, d2[:], start=True, stop=False)
            nc.tensor.matmul(dp3[:], wd3b, d2b[:], start=False, stop=True)
            y = kp.tile([OUT, BC], FP, tag="y")
            nc.vector.tensor_scalar_add(y[:], dp3[:], bd3)
            nc.sync.dma_start(y_d[:], y[:])

    nc.compile()
    return nc


_NC_CACHE = None


def _get_nc():
    global _NC_CACHE
    if _NC_CACHE is None:
        _NC_CACHE = _build_bass()
    return _NC_CACHE


def _prep_core_inputs(inputs):
    """Host-side sharding + layout prep. Returns list of per-core in_maps."""
    x = np.asarray(inputs["x"], np.float32)
    rnn = np.asarray(inputs["rnn_input"], np.float32)
    deltas = np.asarray(inputs["deltas"], np.float32)[..., 0]     # [B,T]
    lengths = np.asarray(inputs["lengths"], np.int64)
    idx = np.clip(lengths - 1, 0, T - 1)                          # [B]

    # modified deltas: d for t<idx, d^2 at t==idx, 0 after
    tt = np.arange(T)[None, :]
    dmod = np.where(tt < idx[:, None], deltas,
                    np.where(tt == idx[:, None], deltas * deltas, 0.0)
                    ).astype(np.float32)

    # gate order [i|f|o|g]; within each gate, z order permuted to [h0|x]
    b = (np.asarray(inputs["bih"], np.float32)
         + np.asarray(inputs["bhh"], np.float32))
    perm_z = np.concatenate([np.arange(SD, Z), np.arange(0, SD)])
    gate_perm = np.concatenate([np.arange(0, Z), np.arange(Z, 2 * Z),
                                np.arange(3 * Z, 4 * Z), np.arange(2 * Z, 3 * Z)])
    col_perm = np.concatenate([gate_perm[blk * Z + perm_z] for blk in range(4)])
    wih = np.asarray(inputs["Wih"], np.float32)[:, col_perm]
    whh = np.asarray(inputs["Whh"], np.float32)[np.ix_(perm_z, col_perm)]
    bih_aug = b[col_perm][None, :]                                # [1, 288]
    wih_aug = np.concatenate([wih, bih_aug], axis=0)              # [6, 288]

    ones = np.ones((B, T, 1), np.float32)
    u_aug = np.concatenate([rnn, ones], axis=2)                   # [B, T, 6]

    wd2 = np.asarray(inputs["Wd2"], np.float32)
    wd3 = np.asarray(inputs["Wd3"], np.float32)
    wd1p = np.asarray(inputs["Wd1"], np.float32)[perm_z]
    constsf = {
        "we1": np.asarray(inputs["We1"], np.float32),
        "we2": np.asarray(inputs["We2"], np.float32),
        "we3": np.asarray(inputs["We3"], np.float32),
        "be1": np.asarray(inputs["be1"], np.float32).reshape(ENC_H, 1),
        "be2": np.asarray(inputs["be2"], np.float32).reshape(ENC_H, 1),
        "be3": np.asarray(inputs["be3"], np.float32).reshape(CRS, 1),
        "bd1": np.asarray(inputs["bd1"], np.float32)[0:128].reshape(128, 1),
        "bd1b": np.asarray(inputs["bd1"], np.float32)[128:].reshape(16, 1),
        "bd2": np.asarray(inputs["bd2"], np.float32)[0:128].reshape(128, 1),
        "bd2b": np.asarray(inputs["bd2"], np.float32)[128:].reshape(16, 1),
        "bd3": np.asarray(inputs["bd3"], np.float32).reshape(OUT, 1),
        "ident": np.eye(Z, dtype=np.float32),
    }
    constsb = {
        "wih": wih_aug,
        "whh": whh,
        "wd1": wd1p,
        "wd2a": wd2[0:128],
        "wd2b": wd2[128:DEC_H],
        "wd3a": wd3[0:128],
        "wd3b": wd3[128:DEC_H],
    }

    base_packf = np.zeros((128, PACKF_COLS), np.float32)
    for name, arr in constsf.items():
        r, c0, c = _PACKF[name]
        assert arr.shape == (r, c), (name, arr.shape, (r, c))
        base_packf[0:r, c0:c0 + c] = arr
    base_packb = np.zeros((128, PACKB_COLS), ml_dtypes.bfloat16)
    for name, arr in constsb.items():
        r, c0, c = _PACKB[name]
        assert arr.shape == (r, c), (name, arr.shape, (r, c))
        base_packb[0:r, c0:c0 + c] = arr.astype(ml_dtypes.bfloat16)

    in_maps = []
    for k in range(NCORES):
        rows = slice(k * BC, (k + 1) * BC)
        p = base_packf.copy()
        r, c0, c = _PACKF["xfm"]
        p[0:r, c0:c0 + c] = x[rows].T
        r, c0, c = _PACKF["dmod"]
        p[0:r, c0:c0 + c] = dmod[rows]
        m = {
            "packf": p,
            "packb": base_packb,
            "u": np.ascontiguousarray(u_aug[rows].transpose(2, 1, 0)
                                      ).reshape(6, T * BC
                                                ).astype(ml_dtypes.bfloat16),
        }
        in_maps.append(m)
    return in_maps


def kernel(**inputs):
    nc = _get_nc()
    in_maps = _prep_core_inputs(inputs)
    res = run_bass_kernel_spmd(nc, in_maps, core_ids=list(range(NCORES)))
    outs = [np.asarray(res.results[k]["y"], np.float32).T
            for k in range(NCORES)]                      # each [BC, OUT]
    return np.ascontiguousarray(np.concatenate(outs, axis=0).astype(np.float32))


# revision 4
# speedup vs baseline: 1.2727x; 1.0220x over previous
"""Trainium2 Bass kernel for nn_CausalFlowModel.

Model: encoder MLP -> discretised-LSTM scan over T=1024 -> interpolated
select at per-sample index -> decoder MLP.

Key algebraic trick (from baseline): the reference computes
    enc[b,t] = (1-d[b,t]) * h[b,t-1] + d[b,t] * h[b,t]
and selects enc[b, idx_b].  Since h[b,t] = h[b,t-1] + d*(h_cand - h[b,t-1]),
enc[b, idx_b] = h[b,idx-1] + d^2*(h_cand - h[b,idx-1]).  So feeding the scan
modified deltas (d for t<idx, d^2 at t==idx, 0 after) makes the final h carry
equal the selected/interpolated value -- no [B,T,Z] materialisation or gather.

v2.1 optimisations:
  * bf16 matmuls AND bf16 elementwise state (DVE 2x mode).  Validated in
    numpy: rel err ~7e-3 vs 2e-2 tolerance.
  * fused elementwise: tanh(g) is written adjacent to persistent [c|h] state
    so ONE tensor_tensor computes [i*gt | f*c]; the discretised update
    X += d*(Xc-X) is tensor_sub + scalar_tensor_tensor per half, with the
    h-half updated FIRST so the transpose -> cast -> next-matmul tail starts
    early and the c-half hides underneath it.
  * dummy bf16 matmul each step keeps the PE HAM clock-gate at 2.4 GHz
    (otherwise the PE runs its sparse per-step matmuls at 1.2 GHz).
  * PSUM pools are scoped: encoder/decoder banks are freed during the scan.

Sharding: data-parallel, batch 512 -> 8 cores x 64.
"""

import numpy as np
import ml_dtypes

import concourse.bass as bass
import concourse.bacc as bacc
import concourse.tile as tile
from concourse import mybir
from concourse.bass_utils import run_bass_kernel_spmd

B, T = 512, 1024
SD, CD = 8, 4
CRS = 64
Z = CRS + SD            # 72
G4 = 4 * Z              # 288
ENC_H = 128
DEC_H = 2 * Z           # 144
OUT = 8
NCORES = 8
BC = B // NCORES        # 64 batch per core

FP = mybir.dt.float32
BF = mybir.dt.bfloat16
U_CHUNK = 128           # time steps per u-DMA chunk

# fp32 packed-constants column layout: name -> (rows, col_off, cols)
_PACKF = {}
_pcf = 0
for _name, _r, _c in [
    ("we1", SD, ENC_H), ("we2", ENC_H, ENC_H), ("we3", ENC_H, CRS),
    ("be1", ENC_H, 1), ("be2", ENC_H, 1), ("be3", CRS, 1),
    ("bd1", 128, 1), ("bd1b", 16, 1), ("bd2", 128, 1), ("bd2b", 16, 1),
    ("bd3", OUT, 1),
    ("ident", Z, Z), ("xfm", SD, BC), ("dmod", BC, T),
]:
    _PACKF[_name] = (_r, _pcf, _c)
    _pcf += _c
PACKF_COLS = _pcf

# bf16 packed-constants column layout
_PACKB = {}
_pcb = 0
for _name, _r, _c in [
    ("wih", 6, G4), ("whh", Z, G4),
    ("wd1", Z, DEC_H), ("wd2a", 128, DEC_H), ("wd2b", 16, DEC_H),
    ("wd3a", 128, OUT), ("wd3b", 16, OUT),
    ("identb", Z, Z),
]:
    _PACKB[_name] = (_r, _pcb, _c)
    _pcb += _c
PACKB_COLS = _pcb


def _build_bass():
    nc = bacc.Bacc("TRN2", target_bir_lowering=False, debug=False)

    packf_d = nc.declare_dram_parameter("packf", [128, PACKF_COLS], FP,
                                        isOutput=False)
    packb_d = nc.declare_dram_parameter("packb", [128, PACKB_COLS], BF,
                                        isOutput=False)
    u_d = nc.declare_dram_parameter("u", [6, T * BC], BF, isOutput=False)
    y_d = nc.declare_dram_parameter("y", [OUT, BC], FP, isOutput=True)

    TANH = mybir.ActivationFunctionType.Tanh
    SIG = mybir.ActivationFunctionType.Sigmoid
    MULT = mybir.AluOpType.mult
    ADD = mybir.AluOpType.add

    with tile.TileContext(nc) as tc:
        with (
            tc.tile_pool(name="w", bufs=1) as wp,
            tc.tile_pool(name="state", bufs=1) as sp,
            tc.tile_pool(name="u", bufs=2) as up,
            tc.tile_pool(name="work", bufs=2) as kp,
        ):
            packf = wp.tile([128, PACKF_COLS], FP, name="packf_sb",
                            tag="packf_sb")
            nc.gpsimd.dma_start(packf[:], packf_d[:])
            packb = wp.tile([128, PACKB_COLS], BF, name="packb_sb",
                            tag="packb_sb")
            nc.gpsimd.dma_start(packb[:], packb_d[:])

            def pkf(name):
                r, c0, c = _PACKF[name]
                return packf[0:r, c0:c0 + c]

            def pkb(name):
                r, c0, c = _PACKB[name]
                return packb[0:r, c0:c0 + c]

            wih, whh = pkb("wih"), pkb("whh")
            wd1, wd2a, wd2b = pkb("wd1"), pkb("wd2a"), pkb("wd2b")
            wd3a, wd3b = pkb("wd3a"), pkb("wd3b")
            identb = pkb("identb")
            we1, we2, we3 = pkf("we1"), pkf("we2"), pkf("we3")
            be1, be2, be3 = pkf("be1"), pkf("be2"), pkf("be3")
            bd1, bd1b, bd2, bd2b, bd3 = (pkf("bd1"), pkf("bd1b"), pkf("bd2"),
                                         pkf("bd2b"), pkf("bd3"))
            ident, xfm, dmod = pkf("ident"), pkf("xfm"), pkf("dmod")

            # ---- persistent state ----
            # W2 = [gt | c | h] batch-major bf16; gt rewritten each step so
            # the pair-product [i*gt | f*c] is a single tensor_tensor.
            W2 = sp.tile([BC, 3 * Z], BF)
            S = sp.tile([BC, 3 * Z], BF)      # sigmoid(i,f,o)
            h_fmB = sp.tile([Z, BC], BF)      # feature-major h (bf16, lhsT)

            # ---- encoder MLP (feature-major, fp32) -> z0 ----
            h_fm32 = sp.tile([Z, BC], FP)
            with tc.tile_pool(name="eps", bufs=2, space="PSUM") as ep_pool:
                ep1 = ep_pool.tile([ENC_H, BC], FP, tag="mlp")
                nc.tensor.matmul(ep1[:], we1, xfm, start=True, stop=True)
                e1 = kp.tile([ENC_H, BC], FP, tag="enc")
                nc.scalar.activation(e1[:], ep1[:], TANH, bias=be1)
                ep2 = ep_pool.tile([ENC_H, BC], FP, tag="mlp")
                nc.tensor.matmul(ep2[:], we2, e1[:], start=True, stop=True)
                e2 = kp.tile([ENC_H, BC], FP, tag="enc")
                nc.scalar.activation(e2[:], ep2[:], TANH, bias=be2)
                ep3 = ep_pool.tile([CRS, BC], FP, tag="mlp")
                nc.tensor.matmul(ep3[:], we3, e2[:], start=True, stop=True)
                # z0_fm, permuted layout [h0|x]: rows 0:64 = h0, 64:72 = x^T
                nc.vector.tensor_scalar_add(h_fm32[0:CRS, :], ep3[:], be3)
                nc.vector.tensor_copy(h_fm32[CRS:Z, :], xfm)
                nc.vector.tensor_copy(h_fmB[:], h_fm32[:])

                # batch-major init: c0 = 0, h0 = transpose(z0_fm)
                nc.vector.memset(W2[:, Z:2 * Z], 0.0)
                zt_ps = ep_pool.tile([BC, Z], FP, tag="mlp")
                nc.tensor.transpose(zt_ps[:], h_fm32[:], ident)
                nc.vector.tensor_copy(W2[:, 2 * Z:3 * Z], zt_ps[:])

            # ---- the scan ----
            with (
                tc.tile_pool(name="ps", bufs=2, space="PSUM") as pp,
                tc.tile_pool(name="pst", bufs=2, space="PSUM") as pt,
                tc.tile_pool(name="psw", bufs=1, space="PSUM") as pw,
            ):
                n_chunks = T // U_CHUNK
                for ci in range(n_chunks):
                    u_sb = up.tile([6, U_CHUNK * BC], BF, tag="u")
                    nc.gpsimd.dma_start(
                        u_sb[:],
                        u_d[:, ci * U_CHUNK * BC:(ci + 1) * U_CHUNK * BC])
                    for tl in range(U_CHUNK):
                        t = ci * U_CHUNK + tl
                        u_t = u_sb[:, tl * BC:(tl + 1) * BC]
                        d_t = dmod[:, t:t + 1]

                        gp = pp.tile([BC, G4], FP, tag="gates")
                        nc.tensor.matmul(gp[:], u_t, wih, start=True,
                                         stop=False)
                        nc.tensor.matmul(gp[:], h_fmB[:], whh, start=False,
                                         stop=True)

                        # gt into W2[:,0:72]; sig(i,f,o) into S
                        nc.scalar.activation(W2[:, 0:Z], gp[:, 3 * Z:G4],
                                             TANH)
                        nc.scalar.activation(S[:, 0:3 * Z], gp[:, 0:3 * Z],
                                             SIG)

                        P = kp.tile([BC, 2 * Z], BF, tag="P")
                        nc.vector.tensor_mul(P[:], S[:, 0:2 * Z],
                                             W2[:, 0:2 * Z])
                        Xc = kp.tile([BC, 2 * Z], BF, tag="Xc")
                        nc.vector.tensor_add(Xc[:, 0:Z], P[:, 0:Z],
                                             P[:, Z:2 * Z])
                        th = kp.tile([BC, Z], BF, tag="th")
                        nc.scalar.activation(th[:], Xc[:, 0:Z], TANH)
                        nc.vector.tensor_mul(Xc[:, Z:2 * Z],
                                             S[:, 2 * Z:3 * Z], th[:])

                        # h-half of the update first: the transpose ->
                        # cast -> next matmul tail depends only on h.
                        dXh = kp.tile([BC, Z], BF, tag="dXh")
                        nc.vector.tensor_sub(dXh[:], Xc[:, Z:2 * Z],
                                             W2[:, 2 * Z:3 * Z])
                        nc.vector.scalar_tensor_tensor(
                            W2[:, 2 * Z:3 * Z], dXh[:], d_t,
                            W2[:, 2 * Z:3 * Z], MULT, ADD)

                        hT = pt.tile([Z, BC], BF, tag="hT")
                        nc.tensor.transpose(hT[:], W2[:, 2 * Z:3 * Z],
                                            identb[0:BC, 0:BC])
                        nc.vector.tensor_copy(h_fmB[:], hT[:])

                        # c-half hides under the transpose/matmul tail
                        dXc = kp.tile([BC, Z], BF, tag="dXc")
                        nc.vector.tensor_sub(dXc[:], Xc[:, 0:Z],
                                             W2[:, Z:2 * Z])
                        nc.vector.scalar_tensor_tensor(
                            W2[:, Z:2 * Z], dXc[:], d_t,
                            W2[:, Z:2 * Z], MULT, ADD)

                        # dummy matmul keeps the PE HAM clock-gate warm
                        warm = pw.tile([BC, 512], FP, tag="warm")
                        nc.tensor.matmul(warm[:], packb[0:Z, 0:BC],
                                         packb[0:Z, 0:512], start=True,
                                         stop=True)

                # ---- decoder MLP on sel = final h (feature-major) ----
                dp1 = pp.tile([128, BC], FP, tag="gates")
                nc.tensor.matmul(dp1[:], wd1[:, 0:128], h_fmB[:], start=True,
                                 stop=True)
                dp1b = pt.tile([16, BC], FP, tag="hT")
                nc.tensor.matmul(dp1b[:], wd1[:, 128:DEC_H], h_fmB[:],
                                 start=True, stop=True)
                d1 = kp.tile([128, BC], BF, tag="dec")
                nc.scalar.activation(d1[:], dp1[:], TANH, bias=bd1)
                d1b = kp.tile([16, BC], BF, tag="decb")
                nc.scalar.activation(d1b[:], dp1b[:], TANH, bias=bd1b)

                dp2 = pp.tile([128, BC], FP, tag="gates")
                nc.tensor.matmul(dp2[:], wd2a[:, 0:128], d1[:], start=True,
                                 stop=False)
                nc.tensor.matmul(dp2[:], wd2b[:, 0:128], d1b[:], start=False,
                                 stop=True)
                dp2b = pt.tile([16, BC], FP, tag="hT")
                nc.tensor.matmul(dp2b[:], wd2a[:, 128:DEC_H], d1[:],
                                 start=True, stop=False)
                nc.tensor.matmul(dp2b[:], wd2b[:, 128:DEC_H], d1b[:],
                                 start=False, stop=True)
                d2 = kp.tile([128, BC], BF, tag="dec")
                nc.scalar.activation(d2[:], dp2[:], TANH, bias=bd2)
                d2b = kp.tile([16, BC], BF, tag="decb")
                nc.scalar.activation(d2b[:], dp2b[:], TANH, bias=bd2b)

                dp3 = pp.tile([OUT, BC], FP, tag="gates")
                nc.tensor.matmul(dp3[:], wd3a, d2[:], start=True, stop=False)
                nc.tensor.matmul(dp3[:], wd3b, d2b[:], start=False, stop=True)
                y = kp.tile([OUT, BC], FP, tag="y")
                nc.vector.tensor_scalar_add(y[:], dp3[:], bd3)
                nc.sync.dma_start(y_d[:], y[:])

    nc.compile()
    return nc


_NC_CACHE = None


def _get_nc():
    global _NC_CACHE
    if _NC_CACHE is None:
        _NC_CACHE = _build_bass()
    return _NC_CACHE


def _prep_core_inputs(inputs):
    """Host-side sharding + layout prep. Returns list of per-core in_maps."""
    x = np.asarray(inputs["x"], np.float32)
    rnn = np.asarray(inputs["rnn_input"], np.float32)
    deltas = np.asarray(inputs["deltas"], np.float32)[..., 0]     # [B,T]
    lengths = np.asarray(inputs["lengths"], np.int64)
    idx = np.clip(lengths - 1, 0, T - 1)                          # [B]

    # modified deltas: d for t<idx, d^2 at t==idx, 0 after
    tt = np.arange(T)[None, :]
    dmod = np.where(tt < idx[:, None], deltas,
                    np.where(tt == idx[:, None], deltas * deltas, 0.0)
                    ).astype(np.float32)

    # gate order [i|f|o|g]; within each gate, z order permuted to [h0|x]
    b = (np.asarray(inputs["bih"], np.float32)
         + np.asarray(inputs["bhh"], np.float32))
    perm_z = np.concatenate([np.arange(SD, Z), np.arange(0, SD)])
    gate_perm = np.concatenate([np.arange(0, Z), np.arange(Z, 2 * Z),
                                np.arange(3 * Z, 4 * Z), np.arange(2 * Z, 3 * Z)])
    col_perm = np.concatenate([gate_perm[blk * Z + perm_z] for blk in range(4)])
    wih = np.asarray(inputs["Wih"], np.float32)[:, col_perm]
    whh = np.asarray(inputs["Whh"], np.float32)[np.ix_(perm_z, col_perm)]
    bih_aug = b[col_perm][None, :]                                # [1, 288]
    wih_aug = np.concatenate([wih, bih_aug], axis=0)              # [6, 288]

    ones = np.ones((B, T, 1), np.float32)
    u_aug = np.concatenate([rnn, ones], axis=2)                   # [B, T, 6]

    wd2 = np.asarray(inputs["Wd2"], np.float32)
    wd3 = np.asarray(inputs["Wd3"], np.float32)
    wd1p = np.asarray(inputs["Wd1"], np.float32)[perm_z]
    constsf = {
        "we1": np.asarray(inputs["We1"], np.float32),
        "we2": np.asarray(inputs["We2"], np.float32),
        "we3": np.asarray(inputs["We3"], np.float32),
        "be1": np.asarray(inputs["be1"], np.float32).reshape(ENC_H, 1),
        "be2": np.asarray(inputs["be2"], np.float32).reshape(ENC_H, 1),
        "be3": np.asarray(inputs["be3"], np.float32).reshape(CRS, 1),
        "bd1": np.asarray(inputs["bd1"], np.float32)[0:128].reshape(128, 1),
        "bd1b": np.asarray(inputs["bd1"], np.float32)[128:].reshape(16, 1),
        "bd2": np.asarray(inputs["bd2"], np.float32)[0:128].reshape(128, 1),
        "bd2b": np.asarray(inputs["bd2"], np.float32)[128:].reshape(16, 1),
        "bd3": np.asarray(inputs["bd3"], np.float32).reshape(OUT, 1),
        "ident": np.eye(Z, dtype=np.float32),
    }
    constsb = {
        "wih": wih_aug,
        "whh": whh,
        "wd1": wd1p,
        "wd2a": wd2[0:128],
        "wd2b": wd2[128:DEC_H],
        "wd3a": wd3[0:128],
        "wd3b": wd3[128:DEC_H],
        "identb": np.eye(Z, dtype=np.float32),
    }

    base_packf = np.zeros((128, PACKF_COLS), np.float32)
    for name, arr in constsf.items():
        r, c0, c = _PACKF[name]
        assert arr.shape == (r, c), (name, arr.shape, (r, c))
        base_packf[0:r, c0:c0 + c] = arr
    base_packb = np.zeros((128, PACKB_COLS), ml_dtypes.bfloat16)
    for name, arr in constsb.items():
        r, c0, c = _PACKB[name]
        assert arr.shape == (r, c), (name, arr.shape, (r, c))
        base_packb[0:r, c0:c0 + c] = arr.astype(ml_dtypes.bfloat16)

    in_maps = []
    for k in range(NCORES):
        rows = slice(k * BC, (k + 1) * BC)
        p = base_packf.copy()
        r, c0, c = _PACKF["xfm"]
        p[0:r, c0:c0 + c] = x[rows].T
        r, c0, c = _PACKF["dmod"]
        p[0:r, c0:c0 + c] = dmod[rows]
        m = {
            "packf": p,
            "packb": base_packb,
            "u": np.ascontiguousarray(u_aug[rows].transpose(2, 1, 0)
                                      ).reshape(6, T * BC
                                                ).astype(ml_dtypes.bfloat16),
        }
        in_maps.append(m)
    return in_maps


def kernel(**inputs):
    nc = _get_nc()
    in_maps = _prep_core_inputs(inputs)
    res = run_bass_kernel_spmd(nc, in_maps, core_ids=list(range(NCORES)))
    outs = [np.asarray(res.results[k]["y"], np.float32).T
            for k in range(NCORES)]                      # each [BC, OUT]
    return np.ascontiguousarray(np.concatenate(outs, axis=0).astype(np.float32))


# revision 10
# speedup vs baseline: 1.3053x; 1.0256x over previous
"""Trainium2 Bass kernel for nn_CausalFlowModel.

Model: encoder MLP -> discretised-LSTM scan over T=1024 -> interpolated
select at per-sample index -> decoder MLP.

Key algebraic trick (from baseline): the reference computes
    enc[b,t] = (1-d[b,t]) * h[b,t-1] + d[b,t] * h[b,t]
and selects enc[b, idx_b].  Since h[b,t] = h[b,t-1] + d*(h_cand - h[b,t-1]),
enc[b, idx_b] = h[b,idx-1] + d^2*(h_cand - h[b,idx-1]).  So feeding the scan
modified deltas (d for t<idx, d^2 at t==idx, 0 after) makes the final h carry
equal the selected/interpolated value -- no [B,T,Z] materialisation or gather.

v2.1 optimisations:
  * bf16 matmuls AND bf16 elementwise state (DVE 2x mode).  Validated in
    numpy: rel err ~7e-3 vs 2e-2 tolerance.
  * fused elementwise: tanh(g) is written adjacent to persistent [c|h] state
    so ONE tensor_tensor computes [i*gt | f*c]; the discretised update
    X += d*(Xc-X) is tensor_sub + scalar_tensor_tensor per half, with the
    h-half updated FIRST so the transpose -> cast -> next-matmul tail starts
    early and the c-half hides underneath it.
  * dummy bf16 matmul each step keeps the PE HAM clock-gate at 2.4 GHz
    (otherwise the PE runs its sparse per-step matmuls at 1.2 GHz).
  * PSUM pools are scoped: encoder/decoder banks are freed during the scan.

Sharding: data-parallel, batch 512 -> 8 cores x 64.
"""

import numpy as np
import ml_dtypes

import concourse.bass as bass
import concourse.bacc as bacc
import concourse.tile as tile
from concourse import mybir
from concourse.bass_utils import run_bass_kernel_spmd

B, T = 512, 1024
SD, CD = 8, 4
CRS = 64
Z = CRS + SD            # 72
G4 = 4 * Z              # 288
ENC_H = 128
DEC_H = 2 * Z           # 144
OUT = 8
NCORES = 8
BC = B // NCORES        # 64 batch per core

FP = mybir.dt.float32
BF = mybir.dt.bfloat16
U_CHUNK = 128           # time steps per u-DMA chunk

# fp32 packed-constants column layout: name -> (rows, col_off, cols)
_PACKF = {}
_pcf = 0
for _name, _r, _c in [
    ("we1", SD, ENC_H), ("we2", ENC_H, ENC_H), ("we3", ENC_H, CRS),
    ("be1", ENC_H, 1), ("be2", ENC_H, 1), ("be3", CRS, 1),
    ("bd1", 128, 1), ("bd1b", 16, 1), ("bd2", 128, 1), ("bd2b", 16, 1),
    ("bd3", OUT, 1),
    ("ident", Z, Z), ("xfm", SD, BC), ("dmod", BC, T),
]:
    _PACKF[_name] = (_r, _pcf, _c)
    _pcf += _c
PACKF_COLS = _pcf

# bf16 packed-constants column layout
_PACKB = {}
_pcb = 0
for _name, _r, _c in [
    ("wih", 6, G4), ("whh", Z, G4),
    ("wd1", Z, DEC_H), ("wd2a", 128, DEC_H), ("wd2b", 16, DEC_H),
    ("wd3a", 128, OUT), ("wd3b", 16, OUT),
    ("identb", Z, Z),
]:
    _PACKB[_name] = (_r, _pcb, _c)
    _pcb += _c
PACKB_COLS = _pcb


def _build_bass():
    nc = bacc.Bacc("TRN2", target_bir_lowering=False, debug=False)

    packf_d = nc.declare_dram_parameter("packf", [128, PACKF_COLS], FP,
                                        isOutput=False)
    packb_d = nc.declare_dram_parameter("packb", [128, PACKB_COLS], BF,
                                        isOutput=False)
    u_d = nc.declare_dram_parameter("u", [6, T * BC], BF, isOutput=False)
    y_d = nc.declare_dram_parameter("y", [OUT, BC], FP, isOutput=True)

    TANH = mybir.ActivationFunctionType.Tanh
    SIG = mybir.ActivationFunctionType.Sigmoid
    MULT = mybir.AluOpType.mult
    ADD = mybir.AluOpType.add

    with tile.TileContext(nc) as tc:
        with (
            tc.tile_pool(name="w", bufs=1) as wp,
            tc.tile_pool(name="state", bufs=1) as sp,
            tc.tile_pool(name="u", bufs=2) as up,
            tc.tile_pool(name="work", bufs=2) as kp,
        ):
            packf = wp.tile([128, PACKF_COLS], FP, name="packf_sb",
                            tag="packf_sb")
            nc.gpsimd.dma_start(packf[:], packf_d[:])
            packb = wp.tile([128, PACKB_COLS], BF, name="packb_sb",
                            tag="packb_sb")
            nc.gpsimd.dma_start(packb[:], packb_d[:])

            def pkf(name):
                r, c0, c = _PACKF[name]
                return packf[0:r, c0:c0 + c]

            def pkb(name):
                r, c0, c = _PACKB[name]
                return packb[0:r, c0:c0 + c]

            wih, whh = pkb("wih"), pkb("whh")
            wd1, wd2a, wd2b = pkb("wd1"), pkb("wd2a"), pkb("wd2b")
            wd3a, wd3b = pkb("wd3a"), pkb("wd3b")
            identb = pkb("identb")
            we1, we2, we3 = pkf("we1"), pkf("we2"), pkf("we3")
            be1, be2, be3 = pkf("be1"), pkf("be2"), pkf("be3")
            bd1, bd1b, bd2, bd2b, bd3 = (pkf("bd1"), pkf("bd1b"), pkf("bd2"),
                                         pkf("bd2b"), pkf("bd3"))
            ident, xfm, dmod = pkf("ident"), pkf("xfm"), pkf("dmod")

            # ---- persistent state ----
            # W2 = [gt | c | h] batch-major bf16; gt rewritten each step so
            # the pair-product [i*gt | f*c] is a single tensor_tensor.
            W2 = sp.tile([BC, 3 * Z], BF)
            S = sp.tile([BC, 3 * Z], BF)      # sigmoid(i,f,o)
            h_fmB = sp.tile([Z, BC], BF)      # feature-major h (bf16, lhsT)

            # ---- encoder MLP (feature-major, fp32) -> z0 ----
            h_fm32 = sp.tile([Z, BC], FP)
            with tc.tile_pool(name="eps", bufs=2, space="PSUM") as ep_pool:
                ep1 = ep_pool.tile([ENC_H, BC], FP, tag="mlp")
                nc.tensor.matmul(ep1[:], we1, xfm, start=True, stop=True)
                e1 = kp.tile([ENC_H, BC], FP, tag="enc")
                nc.scalar.activation(e1[:], ep1[:], TANH, bias=be1)
                ep2 = ep_pool.tile([ENC_H, BC], FP, tag="mlp")
                nc.tensor.matmul(ep2[:], we2, e1[:], start=True, stop=True)
                e2 = kp.tile([ENC_H, BC], FP, tag="enc")
                nc.scalar.activation(e2[:], ep2[:], TANH, bias=be2)
                ep3 = ep_pool.tile([CRS, BC], FP, tag="mlp")
                nc.tensor.matmul(ep3[:], we3, e2[:], start=True, stop=True)
                # z0_fm, permuted layout [h0|x]: rows 0:64 = h0, 64:72 = x^T
                nc.vector.tensor_scalar_add(h_fm32[0:CRS, :], ep3[:], be3)
                nc.vector.tensor_copy(h_fm32[CRS:Z, :], xfm)
                nc.vector.tensor_copy(h_fmB[:], h_fm32[:])

                # batch-major init: c0 = 0, h0 = transpose(z0_fm)
                nc.vector.memset(W2[:, Z:2 * Z], 0.0)
                zt_ps = ep_pool.tile([BC, Z], FP, tag="mlp")
                nc.tensor.transpose(zt_ps[:], h_fm32[:], ident)
                nc.vector.tensor_copy(W2[:, 2 * Z:3 * Z], zt_ps[:])

            # ---- the scan ----
            with (
                tc.tile_pool(name="ps", bufs=2, space="PSUM") as pp,
                tc.tile_pool(name="pst", bufs=2, space="PSUM") as pt,
            ):
                n_chunks = T // U_CHUNK
                for ci in range(n_chunks):
                    u_sb = up.tile([6, U_CHUNK * BC], BF, tag="u")
                    nc.gpsimd.dma_start(
                        u_sb[:],
                        u_d[:, ci * U_CHUNK * BC:(ci + 1) * U_CHUNK * BC])
                    for tl in range(U_CHUNK):
                        t = ci * U_CHUNK + tl
                        u_t = u_sb[:, tl * BC:(tl + 1) * BC]
                        d_t = dmod[:, t:t + 1]

                        gp = pp.tile([BC, G4], FP, tag="gates")
                        nc.tensor.matmul(gp[:], u_t, wih, start=True,
                                         stop=False)
                        # recurrent part split: g-gate columns first so
                        # tanh(g) starts while the i,f,o matmul runs
                        nc.tensor.matmul(gp[:, 3 * Z:G4], h_fmB[:],
                                         whh[:, 3 * Z:G4], start=False,
                                         stop=True)
                        nc.scalar.activation(W2[:, 0:Z], gp[:, 3 * Z:G4],
                                             TANH)
                        nc.tensor.matmul(gp[:, 0:3 * Z], h_fmB[:],
                                         whh[:, 0:3 * Z], start=False,
                                         stop=True)
                        # sig(i,f) on the critical path; sig(o) only feeds
                        # the hc product after tanh(c'), so it hides
                        nc.scalar.activation(S[:, 0:2 * Z], gp[:, 0:2 * Z],
                                             SIG)
                        nc.scalar.activation(S[:, 2 * Z:3 * Z],
                                             gp[:, 2 * Z:3 * Z], SIG)

                        P = kp.tile([BC, 2 * Z], BF, tag="P")
                        nc.vector.tensor_mul(P[:], S[:, 0:2 * Z],
                                             W2[:, 0:2 * Z])
                        Xc = kp.tile([BC, 2 * Z], BF, tag="Xc")
                        nc.vector.tensor_add(Xc[:, 0:Z], P[:, 0:Z],
                                             P[:, Z:2 * Z])
                        th = kp.tile([BC, Z], BF, tag="th")
                        nc.scalar.activation(th[:], Xc[:, 0:Z], TANH)
                        nc.vector.tensor_mul(Xc[:, Z:2 * Z],
                                             S[:, 2 * Z:3 * Z], th[:])

                        # h-half of the update first: the transpose ->
                        # cast -> next matmul tail depends only on h.
                        dXh = kp.tile([BC, Z], BF, tag="dXh")
                        nc.vector.tensor_sub(dXh[:], Xc[:, Z:2 * Z],
                                             W2[:, 2 * Z:3 * Z])
                        nc.vector.scalar_tensor_tensor(
                            W2[:, 2 * Z:3 * Z], dXh[:], d_t,
                            W2[:, 2 * Z:3 * Z], MULT, ADD)

                        hT = pt.tile([Z, BC], BF, tag="hT")
                        nc.tensor.transpose(hT[:], W2[:, 2 * Z:3 * Z],
                                            identb[0:BC, 0:BC])
                        nc.vector.tensor_copy(h_fmB[:], hT[:])

                        # c-half hides under the transpose/matmul tail
                        dXc = kp.tile([BC, Z], BF, tag="dXc")
                        nc.vector.tensor_sub(dXc[:], Xc[:, 0:Z],
                                             W2[:, Z:2 * Z])
                        nc.vector.scalar_tensor_tensor(
                            W2[:, Z:2 * Z], dXc[:], d_t,
                            W2[:, Z:2 * Z], MULT, ADD)

                # ---- decoder MLP on sel = final h (feature-major) ----
                dp1 = pp.tile([128, BC], FP, tag="gates")
                nc.tensor.matmul(dp1[:], wd1[:, 0:128], h_fmB[:], start=True,
                                 stop=True)
                dp1b = pt.tile([16, BC], FP, tag="hT")
                nc.tensor.matmul(dp1b[:], wd1[:, 128:DEC_H], h_fmB[:],
                                 start=True, stop=True)
                d1 = kp.tile([128, BC], BF, tag="dec")
                nc.scalar.activation(d1[:], dp1[:], TANH, bias=bd1)
                d1b = kp.tile([16, BC], BF, tag="decb")
                nc.scalar.activation(d1b[:], dp1b[:], TANH, bias=bd1b)

                dp2 = pp.tile([128, BC], FP, tag="gates")
                nc.tensor.matmul(dp2[:], wd2a[:, 0:128], d1[:], start=True,
                                 stop=False)
                nc.tensor.matmul(dp2[:], wd2b[:, 0:128], d1b[:], start=False,
                                 stop=True)
                dp2b = pt.tile([16, BC], FP, tag="hT")
                nc.tensor.matmul(dp2b[:], wd2a[:, 128:DEC_H], d1[:],
                                 start=True, stop=False)
                nc.tensor.matmul(dp2b[:], wd2b[:, 128:DEC_H], d1b[:],
                                 start=False, stop=True)
                d2 = kp.tile([128, BC], BF, tag="dec")
                nc.scalar.activation(d2[:], dp2[:], TANH, bias=bd2)
                d2b = kp.tile([16, BC], BF, tag="decb")
                nc.scalar.activation(d2b[:], dp2b[:], TANH, bias=bd2b)

                dp3 = pp.tile([OUT, BC], FP, tag="gates")
                nc.tensor.matmul(dp3[:], wd3a, d2[:], start=True, stop=False)
                nc.tensor.matmul(dp3[:], wd3b, d2b[:], start=False, stop=True)
                y = kp.tile([OUT, BC], FP, tag="y")
                nc.vector.tensor_scalar_add(y[:], dp3[:], bd3)
                nc.sync.dma_start(y_d[:], y[:])

    nc.compile()
    return nc


_NC_CACHE = None


def _get_nc():
    global _NC_CACHE
    if _NC_CACHE is None:
        _NC_CACHE = _build_bass()
    return _NC_CACHE


def _prep_core_inputs(inputs):
    """Host-side sharding + layout prep. Returns list of per-core in_maps."""
    x = np.asarray(inputs["x"], np.float32)
    rnn = np.asarray(inputs["rnn_input"], np.float32)
    deltas = np.asarray(inputs["deltas"], np.float32)[..., 0]     # [B,T]
    lengths = np.asarray(inputs["lengths"], np.int64)
    idx = np.clip(lengths - 1, 0, T - 1)                          # [B]

    # modified deltas: d for t<idx, d^2 at t==idx, 0 after
    tt = np.arange(T)[None, :]
    dmod = np.where(tt < idx[:, None], deltas,
                    np.where(tt == idx[:, None], deltas * deltas, 0.0)
                    ).astype(np.float32)

    # gate order [i|f|o|g]; within each gate, z order permuted to [h0|x]
    b = (np.asarray(inputs["bih"], np.float32)
         + np.asarray(inputs["bhh"], np.float32))
    perm_z = np.concatenate([np.arange(SD, Z), np.arange(0, SD)])
    gate_perm = np.concatenate([np.arange(0, Z), np.arange(Z, 2 * Z),
                                np.arange(3 * Z, 4 * Z), np.arange(2 * Z, 3 * Z)])
    col_perm = np.concatenate([gate_perm[blk * Z + perm_z] for blk in range(4)])
    wih = np.asarray(inputs["Wih"], np.float32)[:, col_perm]
    whh = np.asarray(inputs["Whh"], np.float32)[np.ix_(perm_z, col_perm)]
    bih_aug = b[col_perm][None, :]                                # [1, 288]
    wih_aug = np.concatenate([wih, bih_aug], axis=0)              # [6, 288]

    ones = np.ones((B, T, 1), np.float32)
    u_aug = np.concatenate([rnn, ones], axis=2)                   # [B, T, 6]

    wd2 = np.asarray(inputs["Wd2"], np.float32)
    wd3 = np.asarray(inputs["Wd3"], np.float32)
    wd1p = np.asarray(inputs["Wd1"], np.float32)[perm_z]
    constsf = {
        "we1": np.asarray(inputs["We1"], np.float32),
        "we2": np.asarray(inputs["We2"], np.float32),
        "we3": np.asarray(inputs["We3"], np.float32),
        "be1": np.asarray(inputs["be1"], np.float32).reshape(ENC_H, 1),
        "be2": np.asarray(inputs["be2"], np.float32).reshape(ENC_H, 1),
        "be3": np.asarray(inputs["be3"], np.float32).reshape(CRS, 1),
        "bd1": np.asarray(inputs["bd1"], np.float32)[0:128].reshape(128, 1),
        "bd1b": np.asarray(inputs["bd1"], np.float32)[128:].reshape(16, 1),
        "bd2": np.asarray(inputs["bd2"], np.float32)[0:128].reshape(128, 1),
        "bd2b": np.asarray(inputs["bd2"], np.float32)[128:].reshape(16, 1),
        "bd3": np.asarray(inputs["bd3"], np.float32).reshape(OUT, 1),
        "ident": np.eye(Z, dtype=np.float32),
    }
    constsb = {
        "wih": wih_aug,
        "whh": whh,
        "wd1": wd1p,
        "wd2a": wd2[0:128],
        "wd2b": wd2[128:DEC_H],
        "wd3a": wd3[0:128],
        "wd3b": wd3[128:DEC_H],
        "identb": np.eye(Z, dtype=np.float32),
    }

    base_packf = np.zeros((128, PACKF_COLS), np.float32)
    for name, arr in constsf.items():
        r, c0, c = _PACKF[name]
        assert arr.shape == (r, c), (name, arr.shape, (r, c))
        base_packf[0:r, c0:c0 + c] = arr
    base_packb = np.zeros((128, PACKB_COLS), ml_dtypes.bfloat16)
    for name, arr in constsb.items():
        r, c0, c = _PACKB[name]
        assert arr.shape == (r, c), (name, arr.shape, (r, c))
        base_packb[0:r, c0:c0 + c] = arr.astype(ml_dtypes.bfloat16)

    in_maps = []
    for k in range(NCORES):
        rows = slice(k * BC, (k + 1) * BC)
        p = base_packf.copy()
        r, c0, c = _PACKF["xfm"]
        p[0:r, c0:c0 + c] = x[rows].T
        r, c0, c = _PACKF["dmod"]
        p[0:r, c0:c0 + c] = dmod[rows]
        m = {
            "packf": p,
            "packb": base_packb,
            "u": np.ascontiguousarray(u_aug[rows].transpose(2, 1, 0)
                                      ).reshape(6, T * BC
                                                ).astype(ml_dtypes.bfloat16),
        }
        in_maps.append(m)
    return in_maps


def kernel(**inputs):
    nc = _get_nc()
    in_maps = _prep_core_inputs(inputs)
    res = run_bass_kernel_spmd(nc, in_maps, core_ids=list(range(NCORES)))
    outs = [np.asarray(res.results[k]["y"], np.float32).T
            for k in range(NCORES)]                      # each [BC, OUT]
    return np.ascontiguousarray(np.concatenate(outs, axis=0).astype(np.float32))


# revision 11
# speedup vs baseline: 1.6637x; 1.2746x over previous
"""Trainium2 Bass kernel for nn_CausalFlowModel.

Model: encoder MLP -> discretised-LSTM scan over T=1024 -> interpolated
select at per-sample index -> decoder MLP.

Key algebraic trick (from baseline): the reference computes
    enc[b,t] = (1-d[b,t]) * h[b,t-1] + d[b,t] * h[b,t]
and selects enc[b, idx_b].  Since h[b,t] = h[b,t-1] + d*(h_cand - h[b,t-1]),
enc[b, idx_b] = h[b,idx-1] + d^2*(h_cand - h[b,idx-1]).  So feeding the scan
modified deltas (d for t<idx, d^2 at t==idx, 0 after) makes the final h carry
equal the selected/interpolated value -- no [B,T,Z] materialisation or gather.

v2.1 optimisations:
  * bf16 matmuls AND bf16 elementwise state (DVE 2x mode).  Validated in
    numpy: rel err ~7e-3 vs 2e-2 tolerance.
  * fused elementwise: tanh(g) is written adjacent to persistent [c|h] state
    so ONE tensor_tensor computes [i*gt | f*c]; the discretised update
    X += d*(Xc-X) is tensor_sub + scalar_tensor_tensor per half, with the
    h-half updated FIRST so the transpose -> cast -> next-matmul tail starts
    early and the c-half hides underneath it.
  * dummy bf16 matmul each step keeps the PE HAM clock-gate at 2.4 GHz
    (otherwise the PE runs its sparse per-step matmuls at 1.2 GHz).
  * PSUM pools are scoped: encoder/decoder banks are freed during the scan.

Sharding: data-parallel, batch 512 -> 8 cores x 64.
"""

import numpy as np
import ml_dtypes

import concourse.bass as bass
import concourse.bacc as bacc
import concourse.tile as tile
from concourse import mybir
from concourse.bass_utils import run_bass_kernel_spmd

B, T = 512, 1024
SD, CD = 8, 4
CRS = 64
Z = CRS + SD            # 72
G4 = 4 * Z              # 288
ENC_H = 128
DEC_H = 2 * Z           # 144
OUT = 8
NCORES = 8
BC = B // NCORES        # 64 batch per core

FP = mybir.dt.float32
BF = mybir.dt.bfloat16
U_CHUNK = 128           # time steps per u-DMA chunk

# fp32 packed-constants column layout: name -> (rows, col_off, cols)
_PACKF = {}
_pcf = 0
for _name, _r, _c in [
    ("we1", SD, ENC_H), ("we2", ENC_H, ENC_H), ("we3", ENC_H, CRS),
    ("be1", ENC_H, 1), ("be2", ENC_H, 1), ("be3", CRS, 1),
    ("bd1", 128, 1), ("bd1b", 16, 1), ("bd2", 128, 1), ("bd2b", 16, 1),
    ("bd3", OUT, 1),
    ("ident", Z, Z), ("xfm", SD, BC), ("dmod", BC, T), ("dmod2", BC, T),
]:
    _PACKF[_name] = (_r, _pcf, _c)
    _pcf += _c
PACKF_COLS = _pcf

# bf16 packed-constants column layout
_PACKB = {}
_pcb = 0
for _name, _r, _c in [
    ("wih", 6, G4), ("whh", Z, G4),
    ("wd1", Z, DEC_H), ("wd2a", 128, DEC_H), ("wd2b", 16, DEC_H),
    ("wd3a", 128, OUT), ("wd3b", 16, OUT),
    ("identb", Z, Z),
]:
    _PACKB[_name] = (_r, _pcb, _c)
    _pcb += _c
PACKB_COLS = _pcb


def _build_bass():
    nc = bacc.Bacc("TRN2", target_bir_lowering=False, debug=False)

    packf_d = nc.declare_dram_parameter("packf", [128, PACKF_COLS], FP,
                                        isOutput=False)
    packb_d = nc.declare_dram_parameter("packb", [128, PACKB_COLS], BF,
                                        isOutput=False)
    u_d = nc.declare_dram_parameter("u", [6, T * BC], BF, isOutput=False)
    y_d = nc.declare_dram_parameter("y", [OUT, BC], FP, isOutput=True)

    TANH = mybir.ActivationFunctionType.Tanh
    SIG = mybir.ActivationFunctionType.Sigmoid
    MULT = mybir.AluOpType.mult
    ADD = mybir.AluOpType.add

    with tile.TileContext(nc) as tc:
        with (
            tc.tile_pool(name="w", bufs=1) as wp,
            tc.tile_pool(name="state", bufs=1) as sp,
            tc.tile_pool(name="u", bufs=2) as up,
            tc.tile_pool(name="work", bufs=2) as kp,
        ):
            packf = wp.tile([128, PACKF_COLS], FP, name="packf_sb",
                            tag="packf_sb")
            nc.gpsimd.dma_start(packf[:], packf_d[:])
            packb = wp.tile([128, PACKB_COLS], BF, name="packb_sb",
                            tag="packb_sb")
            nc.gpsimd.dma_start(packb[:], packb_d[:])

            def pkf(name):
                r, c0, c = _PACKF[name]
                return packf[0:r, c0:c0 + c]

            def pkb(name):
                r, c0, c = _PACKB[name]
                return packb[0:r, c0:c0 + c]

            wih, whh = pkb("wih"), pkb("whh")
            wd1, wd2a, wd2b = pkb("wd1"), pkb("wd2a"), pkb("wd2b")
            wd3a, wd3b = pkb("wd3a"), pkb("wd3b")
            identb = pkb("identb")
            we1, we2, we3 = pkf("we1"), pkf("we2"), pkf("we3")
            be1, be2, be3 = pkf("be1"), pkf("be2"), pkf("be3")
            bd1, bd1b, bd2, bd2b, bd3 = (pkf("bd1"), pkf("bd1b"), pkf("bd2"),
                                         pkf("bd2b"), pkf("bd3"))
            ident, xfm, dmod = pkf("ident"), pkf("xfm"), pkf("dmod")
            dmod2 = pkf("dmod2")

            # ---- persistent state ----
            # W2 = [gt | c | h] batch-major bf16; gt rewritten each step so
            # the pair-product [i*gt | f*c] is a single tensor_tensor.
            W2 = sp.tile([BC, 3 * Z], BF)
            S = sp.tile([BC, 3 * Z], BF)      # sigmoid(i,f,o)
            h_fmB = sp.tile([Z, BC], BF)      # feature-major h (bf16, lhsT)
            h1 = sp.tile([BC, Z], BF)         # (1-d_t) * h, precomputed
            c1 = sp.tile([BC, Z], BF)         # (1-d_t) * c, precomputed

            # ---- encoder MLP (feature-major, fp32) -> z0 ----
            h_fm32 = sp.tile([Z, BC], FP)
            with tc.tile_pool(name="eps", bufs=2, space="PSUM") as ep_pool:
                ep1 = ep_pool.tile([ENC_H, BC], FP, tag="mlp")
                nc.tensor.matmul(ep1[:], we1, xfm, start=True, stop=True)
                e1 = kp.tile([ENC_H, BC], FP, tag="enc")
                nc.scalar.activation(e1[:], ep1[:], TANH, bias=be1)
                ep2 = ep_pool.tile([ENC_H, BC], FP, tag="mlp")
                nc.tensor.matmul(ep2[:], we2, e1[:], start=True, stop=True)
                e2 = kp.tile([ENC_H, BC], FP, tag="enc")
                nc.scalar.activation(e2[:], ep2[:], TANH, bias=be2)
                ep3 = ep_pool.tile([CRS, BC], FP, tag="mlp")
                nc.tensor.matmul(ep3[:], we3, e2[:], start=True, stop=True)
                # z0_fm, permuted layout [h0|x]: rows 0:64 = h0, 64:72 = x^T
                nc.vector.tensor_scalar_add(h_fm32[0:CRS, :], ep3[:], be3)
                nc.vector.tensor_copy(h_fm32[CRS:Z, :], xfm)
                nc.vector.tensor_copy(h_fmB[:], h_fm32[:])

                # batch-major init: c0 = 0, h0 = transpose(z0_fm)
                nc.vector.memset(W2[:, Z:2 * Z], 0.0)
                zt_ps = ep_pool.tile([BC, Z], FP, tag="mlp")
                nc.tensor.transpose(zt_ps[:], h_fm32[:], ident)
                nc.vector.tensor_copy(W2[:, 2 * Z:3 * Z], zt_ps[:])
                nc.vector.tensor_scalar_mul(h1[:], W2[:, 2 * Z:3 * Z],
                                            dmod2[:, 0:1])
                nc.vector.memset(c1[:], 0.0)

            # ---- the scan ----
            with (
                tc.tile_pool(name="ps", bufs=2, space="PSUM") as pp,
                tc.tile_pool(name="pst", bufs=2, space="PSUM") as pt,
            ):
                n_chunks = T // U_CHUNK
                for ci in range(n_chunks):
                    u_sb = up.tile([6, U_CHUNK * BC], BF, tag="u")
                    nc.gpsimd.dma_start(
                        u_sb[:],
                        u_d[:, ci * U_CHUNK * BC:(ci + 1) * U_CHUNK * BC])
                    for tl in range(U_CHUNK):
                        t = ci * U_CHUNK + tl
                        u_t = u_sb[:, tl * BC:(tl + 1) * BC]
                        d_t = dmod[:, t:t + 1]

                        gp_g = pp.tile([BC, Z], FP, tag="gatesg")
                        gp_ifo = pp.tile([BC, 3 * Z], FP, tag="gates")
                        nc.tensor.matmul(gp_g[:], u_t, wih[:, 3 * Z:G4],
                                         start=True, stop=False)
                        nc.tensor.matmul(gp_ifo[:], u_t, wih[:, 0:3 * Z],
                                         start=True, stop=False)
                        # recurrent part split: g-gate columns first so
                        # tanh(g) starts while the i,f,o matmul runs
                        nc.tensor.matmul(gp_g[:], h_fmB[:],
                                         whh[:, 3 * Z:G4], start=False,
                                         stop=True)
                        nc.scalar.activation(W2[:, 0:Z], gp_g[:], TANH)
                        nc.tensor.matmul(gp_ifo[:], h_fmB[:],
                                         whh[:, 0:3 * Z], start=False,
                                         stop=True)
                        # sig(i,f) on the critical path; sig(o) only feeds
                        # the hc product after tanh(c'), so it hides
                        nc.scalar.activation(S[:, 0:2 * Z], gp_ifo[:, 0:2 * Z],
                                             SIG)
                        nc.scalar.activation(S[:, 2 * Z:3 * Z],
                                             gp_ifo[:, 2 * Z:3 * Z], SIG)

                        P = kp.tile([BC, 2 * Z], BF, tag="P")
                        nc.vector.tensor_mul(P[:], S[:, 0:2 * Z],
                                             W2[:, 0:2 * Z])
                        Xc = kp.tile([BC, 2 * Z], BF, tag="Xc")
                        nc.vector.tensor_add(Xc[:, 0:Z], P[:, 0:Z],
                                             P[:, Z:2 * Z])
                        th = kp.tile([BC, Z], BF, tag="th")
                        nc.scalar.activation(th[:], Xc[:, 0:Z], TANH)

                        # h = (sig_o * d) * th + (1-d)*h   (h1 precomputed)
                        hd = kp.tile([BC, Z], BF, tag="hd")
                        nc.vector.scalar_tensor_tensor(
                            hd[:], S[:, 2 * Z:3 * Z], d_t, th[:], MULT, MULT)
                        nc.vector.tensor_add(W2[:, 2 * Z:3 * Z], hd[:],
                                             h1[:])

                        hT = pt.tile([Z, BC], BF, tag="hT")
                        nc.tensor.transpose(hT[:], W2[:, 2 * Z:3 * Z],
                                            identb[0:BC, 0:BC])
                        nc.vector.tensor_copy(h_fmB[:], hT[:])

                        # c-half + next-step (1-d)* precomputes all hide
                        # under the transpose/matmul tail
                        cd = kp.tile([BC, Z], BF, tag="cd")
                        nc.vector.tensor_scalar_mul(cd[:], Xc[:, 0:Z], d_t)
                        nc.vector.tensor_add(W2[:, Z:2 * Z], cd[:], c1[:])
                        if t + 1 < T:
                            nc.vector.tensor_scalar_mul(
                                h1[:], W2[:, 2 * Z:3 * Z],
                                dmod2[:, t + 1:t + 2])
                            nc.vector.tensor_scalar_mul(
                                c1[:], W2[:, Z:2 * Z],
                                dmod2[:, t + 1:t + 2])

                # ---- decoder MLP on sel = final h (feature-major) ----
                dp1 = pp.tile([128, BC], FP, tag="gates")
                nc.tensor.matmul(dp1[:], wd1[:, 0:128], h_fmB[:], start=True,
                                 stop=True)
                dp1b = pt.tile([16, BC], FP, tag="hT")
                nc.tensor.matmul(dp1b[:], wd1[:, 128:DEC_H], h_fmB[:],
                                 start=True, stop=True)
                d1 = kp.tile([128, BC], BF, tag="dec")
                nc.scalar.activation(d1[:], dp1[:], TANH, bias=bd1)
                d1b = kp.tile([16, BC], BF, tag="decb")
                nc.scalar.activation(d1b[:], dp1b[:], TANH, bias=bd1b)

                dp2 = pp.tile([128, BC], FP, tag="gates")
                nc.tensor.matmul(dp2[:], wd2a[:, 0:128], d1[:], start=True,
                                 stop=False)
                nc.tensor.matmul(dp2[:], wd2b[:, 0:128], d1b[:], start=False,
                                 stop=True)
                dp2b = pt.tile([16, BC], FP, tag="hT")
                nc.tensor.matmul(dp2b[:], wd2a[:, 128:DEC_H], d1[:],
                                 start=True, stop=False)
                nc.tensor.matmul(dp2b[:], wd2b[:, 128:DEC_H], d1b[:],
                                 start=False, stop=True)
                d2 = kp.tile([128, BC], BF, tag="dec")
                nc.scalar.activation(d2[:], dp2[:], TANH, bias=bd2)
                d2b = kp.tile([16, BC], BF, tag="decb")
                nc.scalar.activation(d2b[:], dp2b[:], TANH, bias=bd2b)

                dp3 = pp.tile([OUT, BC], FP, tag="gates")
                nc.tensor.matmul(dp3[:], wd3a, d2[:], start=True, stop=False)
                nc.tensor.matmul(dp3[:], wd3b, d2b[:], start=False, stop=True)
                y = kp.tile([OUT, BC], FP, tag="y")
                nc.vector.tensor_scalar_add(y[:], dp3[:], bd3)
                nc.sync.dma_start(y_d[:], y[:])

    nc.compile()
    return nc


_NC_CACHE = None


def _get_nc():
    global _NC_CACHE
    if _NC_CACHE is None:
        _NC_CACHE = _build_bass()
    return _NC_CACHE


def _prep_core_inputs(inputs):
    """Host-side sharding + layout prep. Returns list of per-core in_maps."""
    x = np.asarray(inputs["x"], np.float32)
    rnn = np.asarray(inputs["rnn_input"], np.float32)
    deltas = np.asarray(inputs["deltas"], np.float32)[..., 0]     # [B,T]
    lengths = np.asarray(inputs["lengths"], np.int64)
    idx = np.clip(lengths - 1, 0, T - 1)                          # [B]

    # modified deltas: d for t<idx, d^2 at t==idx, 0 after
    tt = np.arange(T)[None, :]
    dmod = np.where(tt < idx[:, None], deltas,
                    np.where(tt == idx[:, None], deltas * deltas, 0.0)
                    ).astype(np.float32)

    # gate order [i|f|o|g]; within each gate, z order permuted to [h0|x]
    b = (np.asarray(inputs["bih"], np.float32)
         + np.asarray(inputs["bhh"], np.float32))
    perm_z = np.concatenate([np.arange(SD, Z), np.arange(0, SD)])
    gate_perm = np.concatenate([np.arange(0, Z), np.arange(Z, 2 * Z),
                                np.arange(3 * Z, 4 * Z), np.arange(2 * Z, 3 * Z)])
    col_perm = np.concatenate([gate_perm[blk * Z + perm_z] for blk in range(4)])
    wih = np.asarray(inputs["Wih"], np.float32)[:, col_perm]
    whh = np.asarray(inputs["Whh"], np.float32)[np.ix_(perm_z, col_perm)]
    bih_aug = b[col_perm][None, :]                                # [1, 288]
    wih_aug = np.concatenate([wih, bih_aug], axis=0)              # [6, 288]

    ones = np.ones((B, T, 1), np.float32)
    u_aug = np.concatenate([rnn, ones], axis=2)                   # [B, T, 6]

    wd2 = np.asarray(inputs["Wd2"], np.float32)
    wd3 = np.asarray(inputs["Wd3"], np.float32)
    wd1p = np.asarray(inputs["Wd1"], np.float32)[perm_z]
    constsf = {
        "we1": np.asarray(inputs["We1"], np.float32),
        "we2": np.asarray(inputs["We2"], np.float32),
        "we3": np.asarray(inputs["We3"], np.float32),
        "be1": np.asarray(inputs["be1"], np.float32).reshape(ENC_H, 1),
        "be2": np.asarray(inputs["be2"], np.float32).reshape(ENC_H, 1),
        "be3": np.asarray(inputs["be3"], np.float32).reshape(CRS, 1),
        "bd1": np.asarray(inputs["bd1"], np.float32)[0:128].reshape(128, 1),
        "bd1b": np.asarray(inputs["bd1"], np.float32)[128:].reshape(16, 1),
        "bd2": np.asarray(inputs["bd2"], np.float32)[0:128].reshape(128, 1),
        "bd2b": np.asarray(inputs["bd2"], np.float32)[128:].reshape(16, 1),
        "bd3": np.asarray(inputs["bd3"], np.float32).reshape(OUT, 1),
        "ident": np.eye(Z, dtype=np.float32),
    }
    constsb = {
        "wih": wih_aug,
        "whh": whh,
        "wd1": wd1p,
        "wd2a": wd2[0:128],
        "wd2b": wd2[128:DEC_H],
        "wd3a": wd3[0:128],
        "wd3b": wd3[128:DEC_H],
        "identb": np.eye(Z, dtype=np.float32),
    }

    base_packf = np.zeros((128, PACKF_COLS), np.float32)
    for name, arr in constsf.items():
        r, c0, c = _PACKF[name]
        assert arr.shape == (r, c), (name, arr.shape, (r, c))
        base_packf[0:r, c0:c0 + c] = arr
    base_packb = np.zeros((128, PACKB_COLS), ml_dtypes.bfloat16)
    for name, arr in constsb.items():
        r, c0, c = _PACKB[name]
        assert arr.shape == (r, c), (name, arr.shape, (r, c))
        base_packb[0:r, c0:c0 + c] = arr.astype(ml_dtypes.bfloat16)

    in_maps = []
    for k in range(NCORES):
        rows = slice(k * BC, (k + 1) * BC)
        p = base_packf.copy()
        r, c0, c = _PACKF["xfm"]
        p[0:r, c0:c0 + c] = x[rows].T
        r, c0, c = _PACKF["dmod"]
        p[0:r, c0:c0 + c] = dmod[rows]
        r, c0, c = _PACKF["dmod2"]
        p[0:r, c0:c0 + c] = 1.0 - dmod[rows]
        m = {
            "packf": p,
            "packb": base_packb,
            "u": np.ascontiguousarray(u_aug[rows].transpose(2, 1, 0)
                                      ).reshape(6, T * BC
                                                ).astype(ml_dtypes.bfloat16),
        }
        in_maps.append(m)
    return in_maps


def kernel(**inputs):
    nc = _get_nc()
    in_maps = _prep_core_inputs(inputs)
    res = run_bass_kernel_spmd(nc, in_maps, core_ids=list(range(NCORES)))
    outs = [np.asarray(res.results[k]["y"], np.float32).T
            for k in range(NCORES)]                      # each [BC, OUT]
    return np.ascontiguousarray(np.concatenate(outs, axis=0).astype(np.float32))


# revision 12
# speedup vs baseline: 1.7005x; 1.0221x over previous
"""Trainium2 Bass kernel for nn_CausalFlowModel.

Model: encoder MLP -> discretised-LSTM scan over T=1024 -> interpolated
select at per-sample index -> decoder MLP.

Key algebraic trick (from baseline): the reference computes
    enc[b,t] = (1-d[b,t]) * h[b,t-1] + d[b,t] * h[b,t]
and selects enc[b, idx_b].  Since h[b,t] = h[b,t-1] + d*(h_cand - h[b,t-1]),
enc[b, idx_b] = h[b,idx-1] + d^2*(h_cand - h[b,idx-1]).  So feeding the scan
modified deltas (d for t<idx, d^2 at t==idx, 0 after) makes the final h carry
equal the selected/interpolated value -- no [B,T,Z] materialisation or gather.

v2.1 optimisations:
  * bf16 matmuls AND bf16 elementwise state (DVE 2x mode).  Validated in
    numpy: rel err ~7e-3 vs 2e-2 tolerance.
  * fused elementwise: tanh(g) is written adjacent to persistent [c|h] state
    so ONE tensor_tensor computes [i*gt | f*c]; the discretised update
    X += d*(Xc-X) is tensor_sub + scalar_tensor_tensor per half, with the
    h-half updated FIRST so the transpose -> cast -> next-matmul tail starts
    early and the c-half hides underneath it.
  * dummy bf16 matmul each step keeps the PE HAM clock-gate at 2.4 GHz
    (otherwise the PE runs its sparse per-step matmuls at 1.2 GHz).
  * PSUM pools are scoped: encoder/decoder banks are freed during the scan.

Sharding: data-parallel, batch 512 -> 8 cores x 64.
"""

import numpy as np
import ml_dtypes

import concourse.bass as bass
import concourse.bacc as bacc
import concourse.tile as tile
from concourse import mybir
from concourse.bass_utils import run_bass_kernel_spmd

B, T = 512, 1024
SD, CD = 8, 4
CRS = 64
Z = CRS + SD            # 72
G4 = 4 * Z              # 288
ENC_H = 128
DEC_H = 2 * Z           # 144
OUT = 8
NCORES = 8
BC = B // NCORES        # 64 batch per core

FP = mybir.dt.float32
BF = mybir.dt.bfloat16
U_CHUNK = 128           # time steps per u-DMA chunk

# fp32 packed-constants column layout: name -> (rows, col_off, cols)
_PACKF = {}
_pcf = 0
for _name, _r, _c in [
    ("we1", SD, ENC_H), ("we2", ENC_H, ENC_H), ("we3", ENC_H, CRS),
    ("be1", ENC_H, 1), ("be2", ENC_H, 1), ("be3", CRS, 1),
    ("bd1", 128, 1), ("bd1b", 16, 1), ("bd2", 128, 1), ("bd2b", 16, 1),
    ("bd3", OUT, 1),
    ("ident", Z, Z), ("xfm", SD, BC), ("dmod", BC, T), ("dmod2", BC, T),
]:
    _PACKF[_name] = (_r, _pcf, _c)
    _pcf += _c
PACKF_COLS = _pcf

# bf16 packed-constants column layout
_PACKB = {}
_pcb = 0
for _name, _r, _c in [
    ("wih", 6, G4), ("whh", Z, G4),
    ("wd1", Z, DEC_H), ("wd2a", 128, DEC_H), ("wd2b", 16, DEC_H),
    ("wd3a", 128, OUT), ("wd3b", 16, OUT),
    ("identb", Z, Z),
]:
    _PACKB[_name] = (_r, _pcb, _c)
    _pcb += _c
PACKB_COLS = _pcb


def _build_bass():
    nc = bacc.Bacc("TRN2", target_bir_lowering=False, debug=False)

    packf_d = nc.declare_dram_parameter("packf", [128, PACKF_COLS], FP,
                                        isOutput=False)
    packb_d = nc.declare_dram_parameter("packb", [128, PACKB_COLS], BF,
                                        isOutput=False)
    u_d = nc.declare_dram_parameter("u", [6, T * BC], BF, isOutput=False)
    y_d = nc.declare_dram_parameter("y", [OUT, BC], FP, isOutput=True)

    TANH = mybir.ActivationFunctionType.Tanh
    SIG = mybir.ActivationFunctionType.Sigmoid
    MULT = mybir.AluOpType.mult
    ADD = mybir.AluOpType.add

    with tile.TileContext(nc) as tc:
        with (
            tc.tile_pool(name="w", bufs=1) as wp,
            tc.tile_pool(name="state", bufs=1) as sp,
            tc.tile_pool(name="u", bufs=2) as up,
            tc.tile_pool(name="work", bufs=2) as kp,
        ):
            packf = wp.tile([128, PACKF_COLS], FP, name="packf_sb",
                            tag="packf_sb")
            nc.gpsimd.dma_start(packf[:], packf_d[:])
            packb = wp.tile([128, PACKB_COLS], BF, name="packb_sb",
                            tag="packb_sb")
            nc.gpsimd.dma_start(packb[:], packb_d[:])

            def pkf(name):
                r, c0, c = _PACKF[name]
                return packf[0:r, c0:c0 + c]

            def pkb(name):
                r, c0, c = _PACKB[name]
                return packb[0:r, c0:c0 + c]

            wih, whh = pkb("wih"), pkb("whh")
            wd1, wd2a, wd2b = pkb("wd1"), pkb("wd2a"), pkb("wd2b")
            wd3a, wd3b = pkb("wd3a"), pkb("wd3b")
            identb = pkb("identb")
            we1, we2, we3 = pkf("we1"), pkf("we2"), pkf("we3")
            be1, be2, be3 = pkf("be1"), pkf("be2"), pkf("be3")
            bd1, bd1b, bd2, bd2b, bd3 = (pkf("bd1"), pkf("bd1b"), pkf("bd2"),
                                         pkf("bd2b"), pkf("bd3"))
            ident, xfm, dmod = pkf("ident"), pkf("xfm"), pkf("dmod")
            dmod2 = pkf("dmod2")

            # ---- persistent state ----
            # W2 = [gt | c | h] batch-major bf16; gt rewritten each step so
            # the pair-product [i*gt | f*c] is a single tensor_tensor.
            W2 = sp.tile([BC, 3 * Z], BF)
            S = sp.tile([BC, 3 * Z], BF)      # sigmoid(i,f,o)
            h_fmB = sp.tile([Z, BC], BF)      # feature-major h (bf16, lhsT)
            h1 = sp.tile([BC, Z], BF)         # (1-d_t) * h, precomputed
            c1 = sp.tile([BC, Z], BF)         # (1-d_t) * c, precomputed

            # ---- encoder MLP (feature-major, fp32) -> z0 ----
            h_fm32 = sp.tile([Z, BC], FP)
            with tc.tile_pool(name="eps", bufs=2, space="PSUM") as ep_pool:
                ep1 = ep_pool.tile([ENC_H, BC], FP, tag="mlp")
                nc.tensor.matmul(ep1[:], we1, xfm, start=True, stop=True)
                e1 = kp.tile([ENC_H, BC], FP, tag="enc")
                nc.scalar.activation(e1[:], ep1[:], TANH, bias=be1)
                ep2 = ep_pool.tile([ENC_H, BC], FP, tag="mlp")
                nc.tensor.matmul(ep2[:], we2, e1[:], start=True, stop=True)
                e2 = kp.tile([ENC_H, BC], FP, tag="enc")
                nc.scalar.activation(e2[:], ep2[:], TANH, bias=be2)
                ep3 = ep_pool.tile([CRS, BC], FP, tag="mlp")
                nc.tensor.matmul(ep3[:], we3, e2[:], start=True, stop=True)
                # z0_fm, permuted layout [h0|x]: rows 0:64 = h0, 64:72 = x^T
                nc.vector.tensor_scalar_add(h_fm32[0:CRS, :], ep3[:], be3)
                nc.vector.tensor_copy(h_fm32[CRS:Z, :], xfm)
                nc.vector.tensor_copy(h_fmB[:], h_fm32[:])

                # batch-major init: c0 = 0, h0 = transpose(z0_fm)
                nc.vector.memset(W2[:, Z:2 * Z], 0.0)
                zt_ps = ep_pool.tile([BC, Z], FP, tag="mlp")
                nc.tensor.transpose(zt_ps[:], h_fm32[:], ident)
                nc.vector.tensor_copy(W2[:, 2 * Z:3 * Z], zt_ps[:])
                nc.vector.tensor_scalar_mul(h1[:], W2[:, 2 * Z:3 * Z],
                                            dmod2[:, 0:1])
                nc.vector.memset(c1[:], 0.0)

            # ---- the scan ----
            with (
                tc.tile_pool(name="ps", bufs=2, space="PSUM") as pp,
                tc.tile_pool(name="pst", bufs=2, space="PSUM") as pt,
            ):
                n_chunks = T // U_CHUNK
                for ci in range(n_chunks):
                    u_sb = up.tile([6, U_CHUNK * BC], BF, tag="u")
                    nc.gpsimd.dma_start(
                        u_sb[:],
                        u_d[:, ci * U_CHUNK * BC:(ci + 1) * U_CHUNK * BC])
                    for tl in range(U_CHUNK):
                        t = ci * U_CHUNK + tl
                        u_t = u_sb[:, tl * BC:(tl + 1) * BC]
                        d_t = dmod[:, t:t + 1]

                        gp_g = pp.tile([BC, Z], FP, tag="gatesg")
                        gp_ifo = pp.tile([BC, 3 * Z], FP, tag="gates")
                        nc.tensor.matmul(gp_g[:], u_t, wih[:, 3 * Z:G4],
                                         start=True, stop=False)
                        nc.tensor.matmul(gp_ifo[:], u_t, wih[:, 0:3 * Z],
                                         start=True, stop=False)
                        # recurrent part split: g-gate columns first so
                        # tanh(g) starts while the i,f,o matmul runs
                        nc.tensor.matmul(gp_g[:], h_fmB[:],
                                         whh[:, 3 * Z:G4], start=False,
                                         stop=True)
                        nc.scalar.activation(W2[:, 0:Z], gp_g[:], TANH)
                        nc.tensor.matmul(gp_ifo[:], h_fmB[:],
                                         whh[:, 0:3 * Z], start=False,
                                         stop=True)
                        # sig(i,f) on the critical path; sig(o) only feeds
                        # the hc product after tanh(c'), so it hides
                        nc.scalar.activation(S[:, 0:2 * Z], gp_ifo[:, 0:2 * Z],
                                             SIG)
                        nc.scalar.activation(S[:, 2 * Z:3 * Z],
                                             gp_ifo[:, 2 * Z:3 * Z], SIG)

                        P = kp.tile([BC, 2 * Z], BF, tag="P")
                        nc.vector.tensor_mul(P[:], S[:, 0:2 * Z],
                                             W2[:, 0:2 * Z])
                        Xc = kp.tile([BC, 2 * Z], BF, tag="Xc")
                        nc.vector.tensor_add(Xc[:, 0:Z], P[:, 0:Z],
                                             P[:, Z:2 * Z])
                        # sig_o * d hides right after the sig_o act,
                        # before tanh(c') completes
                        sod = kp.tile([BC, Z], BF, tag="sod")
                        nc.vector.tensor_scalar_mul(sod[:],
                                                    S[:, 2 * Z:3 * Z], d_t)
                        th = kp.tile([BC, Z], BF, tag="th")
                        nc.scalar.activation(th[:], Xc[:, 0:Z], TANH)

                        # h = (sig_o * d) * th + (1-d)*h   (both precomputed)
                        hd = kp.tile([BC, Z], BF, tag="hd")
                        nc.vector.tensor_mul(hd[:], sod[:], th[:])
                        nc.vector.tensor_add(W2[:, 2 * Z:3 * Z], hd[:],
                                             h1[:])

                        hT = pt.tile([Z, BC], BF, tag="hT")
                        nc.tensor.transpose(hT[:], W2[:, 2 * Z:3 * Z],
                                            identb[0:BC, 0:BC])
                        nc.vector.tensor_copy(h_fmB[:], hT[:])

                        # c-half + next-step (1-d)* precomputes all hide
                        # under the transpose/matmul tail
                        cd = kp.tile([BC, Z], BF, tag="cd")
                        nc.vector.tensor_scalar_mul(cd[:], Xc[:, 0:Z], d_t)
                        nc.vector.tensor_add(W2[:, Z:2 * Z], cd[:], c1[:])
                        if t + 1 < T:
                            nc.vector.tensor_scalar_mul(
                                h1[:], W2[:, 2 * Z:3 * Z],
                                dmod2[:, t + 1:t + 2])
                            nc.vector.tensor_scalar_mul(
                                c1[:], W2[:, Z:2 * Z],
                                dmod2[:, t + 1:t + 2])

                # ---- decoder MLP on sel = final h (feature-major) ----
                dp1 = pp.tile([128, BC], FP, tag="gates")
                nc.tensor.matmul(dp1[:], wd1[:, 0:128], h_fmB[:], start=True,
                                 stop=True)
                dp1b = pt.tile([16, BC], FP, tag="hT")
                nc.tensor.matmul(dp1b[:], wd1[:, 128:DEC_H], h_fmB[:],
                                 start=True, stop=True)
                d1 = kp.tile([128, BC], BF, tag="dec")
                nc.scalar.activation(d1[:], dp1[:], TANH, bias=bd1)
                d1b = kp.tile([16, BC], BF, tag="decb")
                nc.scalar.activation(d1b[:], dp1b[:], TANH, bias=bd1b)

                dp2 = pp.tile([128, BC], FP, tag="gates")
                nc.tensor.matmul(dp2[:], wd2a[:, 0:128], d1[:], start=True,
                                 stop=False)
                nc.tensor.matmul(dp2[:], wd2b[:, 0:128], d1b[:], start=False,
                                 stop=True)
                dp2b = pt.tile([16, BC], FP, tag="hT")
                nc.tensor.matmul(dp2b[:], wd2a[:, 128:DEC_H], d1[:],
                                 start=True, stop=False)
                nc.tensor.matmul(dp2b[:], wd2b[:, 128:DEC_H], d1b[:],
                                 start=False, stop=True)
                d2 = kp.tile([128, BC], BF, tag="dec")
                nc.scalar.activation(d2[:], dp2[:], TANH, bias=bd2)
                d2b = kp.tile([16, BC], BF, tag="decb")
                nc.scalar.activation(d2b[:], dp2b[:], TANH, bias=bd2b)

                dp3 = pp.tile([OUT, BC], FP, tag="gates")
                nc.tensor.matmul(dp3[:], wd3a, d2[:], start=True, stop=False)
                nc.tensor.matmul(dp3[:], wd3b, d2b[:], start=False, stop=True)
                y = kp.tile([OUT, BC], FP, tag="y")
                nc.vector.tensor_scalar_add(y[:], dp3[:], bd3)
                nc.sync.dma_start(y_d[:], y[:])

    nc.compile()
    return nc


_NC_CACHE = None


def _get_nc():
    global _NC_CACHE
    if _NC_CACHE is None:
        _NC_CACHE = _build_bass()
    return _NC_CACHE


def _prep_core_inputs(inputs):
    """Host-side sharding + layout prep. Returns list of per-core in_maps."""
    x = np.asarray(inputs["x"], np.float32)
    rnn = np.asarray(inputs["rnn_input"], np.float32)
    deltas = np.asarray(inputs["deltas"], np.float32)[..., 0]     # [B,T]
    lengths = np.asarray(inputs["lengths"], np.int64)
    idx = np.clip(lengths - 1, 0, T - 1)                          # [B]

    # modified deltas: d for t<idx, d^2 at t==idx, 0 after
    tt = np.arange(T)[None, :]
    dmod = np.where(tt < idx[:, None], deltas,
                    np.where(tt == idx[:, None], deltas * deltas, 0.0)
                    ).astype(np.float32)

    # gate order [i|f|o|g]; within each gate, z order permuted to [h0|x]
    b = (np.asarray(inputs["bih"], np.float32)
         + np.asarray(inputs["bhh"], np.float32))
    perm_z = np.concatenate([np.arange(SD, Z), np.arange(0, SD)])
    gate_perm = np.concatenate([np.arange(0, Z), np.arange(Z, 2 * Z),
                                np.arange(3 * Z, 4 * Z), np.arange(2 * Z, 3 * Z)])
    col_perm = np.concatenate([gate_perm[blk * Z + perm_z] for blk in range(4)])
    wih = np.asarray(inputs["Wih"], np.float32)[:, col_perm]
    whh = np.asarray(inputs["Whh"], np.float32)[np.ix_(perm_z, col_perm)]
    bih_aug = b[col_perm][None, :]                                # [1, 288]
    wih_aug = np.concatenate([wih, bih_aug], axis=0)              # [6, 288]

    ones = np.ones((B, T, 1), np.float32)
    u_aug = np.concatenate([rnn, ones], axis=2)                   # [B, T, 6]

    wd2 = np.asarray(inputs["Wd2"], np.float32)
    wd3 = np.asarray(inputs["Wd3"], np.float32)
    wd1p = np.asarray(inputs["Wd1"], np.float32)[perm_z]
    constsf = {
        "we1": np.asarray(inputs["We1"], np.float32),
        "we2": np.asarray(inputs["We2"], np.float32),
        "we3": np.asarray(inputs["We3"], np.float32),
        "be1": np.asarray(inputs["be1"], np.float32).reshape(ENC_H, 1),
        "be2": np.asarray(inputs["be2"], np.float32).reshape(ENC_H, 1),
        "be3": np.asarray(inputs["be3"], np.float32).reshape(CRS, 1),
        "bd1": np.asarray(inputs["bd1"], np.float32)[0:128].reshape(128, 1),
        "bd1b": np.asarray(inputs["bd1"], np.float32)[128:].reshape(16, 1),
        "bd2": np.asarray(inputs["bd2"], np.float32)[0:128].reshape(128, 1),
        "bd2b": np.asarray(inputs["bd2"], np.float32)[128:].reshape(16, 1),
        "bd3": np.asarray(inputs["bd3"], np.float32).reshape(OUT, 1),
        "ident": np.eye(Z, dtype=np.float32),
    }
    constsb = {
        "wih": wih_aug,
        "whh": whh,
        "wd1": wd1p,
        "wd2a": wd2[0:128],
        "wd2b": wd2[128:DEC_H],
        "wd3a": wd3[0:128],
        "wd3b": wd3[128:DEC_H],
        "identb": np.eye(Z, dtype=np.float32),
    }

    base_packf = np.zeros((128, PACKF_COLS), np.float32)
    for name, arr in constsf.items():
        r, c0, c = _PACKF[name]
        assert arr.shape == (r, c), (name, arr.shape, (r, c))
        base_packf[0:r, c0:c0 + c] = arr
    base_packb = np.zeros((128, PACKB_COLS), ml_dtypes.bfloat16)
    for name, arr in constsb.items():
        r, c0, c = _PACKB[name]
        assert arr.shape == (r, c), (name, arr.shape, (r, c))
        base_packb[0:r, c0:c0 + c] = arr.astype(ml_dtypes.bfloat16)

    in_maps = []
    for k in range(NCORES):
        rows = slice(k * BC, (k + 1) * BC)
        p = base_packf.copy()
        r, c0, c = _PACKF["xfm"]
        p[0:r, c0:c0 + c] = x[rows].T
        r, c0, c = _PACKF["dmod"]
        p[0:r, c0:c0 + c] = dmod[rows]
        r, c0, c = _PACKF["dmod2"]
        p[0:r, c0:c0 + c] = 1.0 - dmod[rows]
        m = {
            "packf": p,
            "packb": base_packb,
            "u": np.ascontiguousarray(u_aug[rows].transpose(2, 1, 0)
                                      ).reshape(6, T * BC
                                                ).astype(ml_dtypes.bfloat16),
        }
        in_maps.append(m)
    return in_maps


def kernel(**inputs):
    nc = _get_nc()
    in_maps = _prep_core_inputs(inputs)
    res = run_bass_kernel_spmd(nc, in_maps, core_ids=list(range(NCORES)))
    outs = [np.asarray(res.results[k]["y"], np.float32).T
            for k in range(NCORES)]                      # each [BC, OUT]
    return np.ascontiguousarray(np.concatenate(outs, axis=0).astype(np.float32))


# revision 13
# speedup vs baseline: 1.7296x; 1.0171x over previous
"""Trainium2 Bass kernel for nn_CausalFlowModel.

Model: encoder MLP -> discretised-LSTM scan over T=1024 -> interpolated
select at per-sample index -> decoder MLP.

Key algebraic trick (from baseline): the reference computes
    enc[b,t] = (1-d[b,t]) * h[b,t-1] + d[b,t] * h[b,t]
and selects enc[b, idx_b].  Since h[b,t] = h[b,t-1] + d*(h_cand - h[b,t-1]),
enc[b, idx_b] = h[b,idx-1] + d^2*(h_cand - h[b,idx-1]).  So feeding the scan
modified deltas (d for t<idx, d^2 at t==idx, 0 after) makes the final h carry
equal the selected/interpolated value -- no [B,T,Z] materialisation or gather.

v2.1 optimisations:
  * bf16 matmuls AND bf16 elementwise state (DVE 2x mode).  Validated in
    numpy: rel err ~7e-3 vs 2e-2 tolerance.
  * fused elementwise: tanh(g) is written adjacent to persistent [c|h] state
    so ONE tensor_tensor computes [i*gt | f*c]; the discretised update
    X += d*(Xc-X) is tensor_sub + scalar_tensor_tensor per half, with the
    h-half updated FIRST so the transpose -> cast -> next-matmul tail starts
    early and the c-half hides underneath it.
  * dummy bf16 matmul each step keeps the PE HAM clock-gate at 2.4 GHz
    (otherwise the PE runs its sparse per-step matmuls at 1.2 GHz).
  * PSUM pools are scoped: encoder/decoder banks are freed during the scan.

Sharding: data-parallel, batch 512 -> 8 cores x 64.
"""

import numpy as np
import ml_dtypes

import concourse.bass as bass
import concourse.bacc as bacc
import concourse.tile as tile
from concourse import mybir
from concourse.bass_utils import run_bass_kernel_spmd

B, T = 512, 1024
SD, CD = 8, 4
CRS = 64
Z = CRS + SD            # 72
G4 = 4 * Z              # 288
ENC_H = 128
DEC_H = 2 * Z           # 144
OUT = 8
NCORES = 8
BC = B // NCORES        # 64 batch per core

FP = mybir.dt.float32
BF = mybir.dt.bfloat16
U_CHUNK = 128           # time steps per u-DMA chunk

# fp32 packed-constants column layout: name -> (rows, col_off, cols)
_PACKF = {}
_pcf = 0
for _name, _r, _c in [
    ("we1", SD, ENC_H), ("we2", ENC_H, ENC_H), ("we3", ENC_H, CRS),
    ("be1", ENC_H, 1), ("be2", ENC_H, 1), ("be3", CRS, 1),
    ("bd1", 128, 1), ("bd1b", 16, 1), ("bd2", 128, 1), ("bd2b", 16, 1),
    ("bd3", OUT, 1),
    ("ident", Z, Z), ("xfm", SD, BC), ("dmod", BC, T), ("dmod2", BC, T),
]:
    _PACKF[_name] = (_r, _pcf, _c)
    _pcf += _c
PACKF_COLS = _pcf

# bf16 packed-constants column layout
_PACKB = {}
_pcb = 0
for _name, _r, _c in [
    ("wih", 6, G4), ("whh", Z, G4),
    ("wd1", Z, DEC_H), ("wd2a", 128, DEC_H), ("wd2b", 16, DEC_H),
    ("wd3a", 128, OUT), ("wd3b", 16, OUT),
    ("identb", Z, Z),
]:
    _PACKB[_name] = (_r, _pcb, _c)
    _pcb += _c
PACKB_COLS = _pcb


def _build_bass():
    nc = bacc.Bacc("TRN2", target_bir_lowering=False, debug=False)

    packf_d = nc.declare_dram_parameter("packf", [128, PACKF_COLS], FP,
                                        isOutput=False)
    packb_d = nc.declare_dram_parameter("packb", [128, PACKB_COLS], BF,
                                        isOutput=False)
    u_d = nc.declare_dram_parameter("u", [6, T * BC], BF, isOutput=False)
    y_d = nc.declare_dram_parameter("y", [OUT, BC], FP, isOutput=True)

    TANH = mybir.ActivationFunctionType.Tanh
    SIG = mybir.ActivationFunctionType.Sigmoid
    MULT = mybir.AluOpType.mult
    ADD = mybir.AluOpType.add

    with tile.TileContext(nc) as tc:
        with (
            tc.tile_pool(name="w", bufs=1) as wp,
            tc.tile_pool(name="state", bufs=1) as sp,
            tc.tile_pool(name="u", bufs=2) as up,
            tc.tile_pool(name="work", bufs=2) as kp,
        ):
            packf = wp.tile([128, PACKF_COLS], FP, name="packf_sb",
                            tag="packf_sb")
            nc.gpsimd.dma_start(packf[:], packf_d[:])
            packb = wp.tile([128, PACKB_COLS], BF, name="packb_sb",
                            tag="packb_sb")
            nc.gpsimd.dma_start(packb[:], packb_d[:])

            def pkf(name):
                r, c0, c = _PACKF[name]
                return packf[0:r, c0:c0 + c]

            def pkb(name):
                r, c0, c = _PACKB[name]
                return packb[0:r, c0:c0 + c]

            wih, whh = pkb("wih"), pkb("whh")
            wd1, wd2a, wd2b = pkb("wd1"), pkb("wd2a"), pkb("wd2b")
            wd3a, wd3b = pkb("wd3a"), pkb("wd3b")
            identb = pkb("identb")
            we1, we2, we3 = pkf("we1"), pkf("we2"), pkf("we3")
            be1, be2, be3 = pkf("be1"), pkf("be2"), pkf("be3")
            bd1, bd1b, bd2, bd2b, bd3 = (pkf("bd1"), pkf("bd1b"), pkf("bd2"),
                                         pkf("bd2b"), pkf("bd3"))
            ident, xfm, dmod = pkf("ident"), pkf("xfm"), pkf("dmod")
            dmod2 = pkf("dmod2")

            # ---- persistent state ----
            # W2 = [gt | c | h] batch-major bf16; gt rewritten each step so
            # the pair-product [i*gt | f*c] is a single tensor_tensor.
            W2 = sp.tile([BC, 3 * Z], BF)
            S = sp.tile([BC, 3 * Z], BF)      # sigmoid(i,f,o)
            h_fmB = sp.tile([Z, BC], BF)      # feature-major h (bf16, lhsT)
            hc1 = sp.tile([BC, 2 * Z], BF)    # [(1-d_t)*c | (1-d_t)*h]

            # ---- encoder MLP (feature-major, fp32) -> z0 ----
            h_fm32 = sp.tile([Z, BC], FP)
            with tc.tile_pool(name="eps", bufs=2, space="PSUM") as ep_pool:
                ep1 = ep_pool.tile([ENC_H, BC], FP, tag="mlp")
                nc.tensor.matmul(ep1[:], we1, xfm, start=True, stop=True)
                e1 = kp.tile([ENC_H, BC], FP, tag="enc")
                nc.scalar.activation(e1[:], ep1[:], TANH, bias=be1)
                ep2 = ep_pool.tile([ENC_H, BC], FP, tag="mlp")
                nc.tensor.matmul(ep2[:], we2, e1[:], start=True, stop=True)
                e2 = kp.tile([ENC_H, BC], FP, tag="enc")
                nc.scalar.activation(e2[:], ep2[:], TANH, bias=be2)
                ep3 = ep_pool.tile([CRS, BC], FP, tag="mlp")
                nc.tensor.matmul(ep3[:], we3, e2[:], start=True, stop=True)
                # z0_fm, permuted layout [h0|x]: rows 0:64 = h0, 64:72 = x^T
                nc.vector.tensor_scalar_add(h_fm32[0:CRS, :], ep3[:], be3)
                nc.vector.tensor_copy(h_fm32[CRS:Z, :], xfm)
                nc.vector.tensor_copy(h_fmB[:], h_fm32[:])

                # batch-major init: c0 = 0, h0 = transpose(z0_fm)
                nc.vector.memset(W2[:, Z:2 * Z], 0.0)
                zt_ps = ep_pool.tile([BC, Z], FP, tag="mlp")
                nc.tensor.transpose(zt_ps[:], h_fm32[:], ident)
                nc.vector.tensor_copy(W2[:, 2 * Z:3 * Z], zt_ps[:])
                nc.vector.tensor_scalar_mul(hc1[:, Z:2 * Z],
                                            W2[:, 2 * Z:3 * Z],
                                            dmod2[:, 0:1])
                nc.vector.memset(hc1[:, 0:Z], 0.0)

            # ---- the scan ----
            with (
                tc.tile_pool(name="ps", bufs=2, space="PSUM") as pp,
                tc.tile_pool(name="pst", bufs=2, space="PSUM") as pt,
            ):
                n_chunks = T // U_CHUNK
                for ci in range(n_chunks):
                    u_sb = up.tile([6, U_CHUNK * BC], BF, tag="u")
                    nc.gpsimd.dma_start(
                        u_sb[:],
                        u_d[:, ci * U_CHUNK * BC:(ci + 1) * U_CHUNK * BC])
                    for tl in range(U_CHUNK):
                        t = ci * U_CHUNK + tl
                        u_t = u_sb[:, tl * BC:(tl + 1) * BC]
                        d_t = dmod[:, t:t + 1]

                        gp_g = pp.tile([BC, Z], FP, tag="gatesg")
                        gp_ifo = pp.tile([BC, 3 * Z], FP, tag="gates")
                        nc.tensor.matmul(gp_g[:], u_t, wih[:, 3 * Z:G4],
                                         start=True, stop=False)
                        nc.tensor.matmul(gp_ifo[:], u_t, wih[:, 0:3 * Z],
                                         start=True, stop=False)
                        # recurrent part split: g-gate columns first so
                        # tanh(g) starts while the i,f,o matmul runs
                        nc.tensor.matmul(gp_g[:], h_fmB[:],
                                         whh[:, 3 * Z:G4], start=False,
                                         stop=True)
                        nc.scalar.activation(W2[:, 0:Z], gp_g[:], TANH)
                        nc.tensor.matmul(gp_ifo[:], h_fmB[:],
                                         whh[:, 0:3 * Z], start=False,
                                         stop=True)
                        # sig(i,f) on the critical path; sig(o) only feeds
                        # the hc product after tanh(c'), so it hides
                        nc.scalar.activation(S[:, 0:2 * Z], gp_ifo[:, 0:2 * Z],
                                             SIG)
                        nc.scalar.activation(S[:, 2 * Z:3 * Z],
                                             gp_ifo[:, 2 * Z:3 * Z], SIG)

                        P = kp.tile([BC, 2 * Z], BF, tag="P")
                        nc.vector.tensor_mul(P[:], S[:, 0:2 * Z],
                                             W2[:, 0:2 * Z])
                        Xc = kp.tile([BC, 2 * Z], BF, tag="Xc")
                        nc.vector.tensor_add(Xc[:, 0:Z], P[:, 0:Z],
                                             P[:, Z:2 * Z])
                        # sig_o * d hides right after the sig_o act,
                        # before tanh(c') completes
                        sod = kp.tile([BC, Z], BF, tag="sod")
                        nc.vector.tensor_scalar_mul(sod[:],
                                                    S[:, 2 * Z:3 * Z], d_t)
                        th = kp.tile([BC, Z], BF, tag="th")
                        nc.scalar.activation(th[:], Xc[:, 0:Z], TANH)

                        # h = (sig_o * d) * th + (1-d)*h   (both precomputed)
                        hd = kp.tile([BC, Z], BF, tag="hd")
                        nc.vector.tensor_mul(hd[:], sod[:], th[:])
                        nc.vector.tensor_add(W2[:, 2 * Z:3 * Z], hd[:],
                                             hc1[:, Z:2 * Z])

                        hT = pt.tile([Z, BC], BF, tag="hT")
                        nc.tensor.transpose(hT[:], W2[:, 2 * Z:3 * Z],
                                            identb[0:BC, 0:BC])
                        nc.vector.tensor_copy(h_fmB[:], hT[:])

                        # c-half + next-step (1-d)* precomputes all hide
                        # under the transpose/matmul tail
                        cd = kp.tile([BC, Z], BF, tag="cd")
                        nc.vector.tensor_scalar_mul(cd[:], Xc[:, 0:Z], d_t)
                        # c-add on GpSimd so DVE is free when tanh finishes
                        nc.gpsimd.tensor_add(W2[:, Z:2 * Z], cd[:],
                                             hc1[:, 0:Z])
                        if t + 1 < T:
                            # one op for both (1-d)* precomputes; reading
                            # W2[c|h] also orders it after the h update so
                            # the scheduler cannot wedge it into the
                            # hd -> hsum window
                            nc.vector.tensor_scalar_mul(
                                hc1[:], W2[:, Z:3 * Z],
                                dmod2[:, t + 1:t + 2])

                # ---- decoder MLP on sel = final h (feature-major) ----
                dp1 = pp.tile([128, BC], FP, tag="gates")
                nc.tensor.matmul(dp1[:], wd1[:, 0:128], h_fmB[:], start=True,
                                 stop=True)
                dp1b = pt.tile([16, BC], FP, tag="hT")
                nc.tensor.matmul(dp1b[:], wd1[:, 128:DEC_H], h_fmB[:],
                                 start=True, stop=True)
                d1 = kp.tile([128, BC], BF, tag="dec")
                nc.scalar.activation(d1[:], dp1[:], TANH, bias=bd1)
                d1b = kp.tile([16, BC], BF, tag="decb")
                nc.scalar.activation(d1b[:], dp1b[:], TANH, bias=bd1b)

                dp2 = pp.tile([128, BC], FP, tag="gates")
                nc.tensor.matmul(dp2[:], wd2a[:, 0:128], d1[:], start=True,
                                 stop=False)
                nc.tensor.matmul(dp2[:], wd2b[:, 0:128], d1b[:], start=False,
                                 stop=True)
                dp2b = pt.tile([16, BC], FP, tag="hT")
                nc.tensor.matmul(dp2b[:], wd2a[:, 128:DEC_H], d1[:],
                                 start=True, stop=False)
                nc.tensor.matmul(dp2b[:], wd2b[:, 128:DEC_H], d1b[:],
                                 start=False, stop=True)
                d2 = kp.tile([128, BC], BF, tag="dec")
                nc.scalar.activation(d2[:], dp2[:], TANH, bias=bd2)
                d2b = kp.tile([16, BC], BF, tag="decb")
                nc.scalar.activation(d2b[:], dp2b[:], TANH, bias=bd2b)

                dp3 = pp.tile([OUT, BC], FP, tag="gates")
                nc.tensor.matmul(dp3[:], wd3a, d2[:], start=True, stop=False)
                nc.tensor.matmul(dp3[:], wd3b, d2b[:], start=False, stop=True)
                y = kp.tile([OUT, BC], FP, tag="y")
                nc.vector.tensor_scalar_add(y[:], dp3[:], bd3)
                nc.sync.dma_start(y_d[:], y[:])

    nc.compile()
    return nc


_NC_CACHE = None


def _get_nc():
    global _NC_CACHE
    if _NC_CACHE is None:
        _NC_CACHE = _build_bass()
    return _NC_CACHE


def _prep_core_inputs(inputs):
    """Host-side sharding + layout prep. Returns list of per-core in_maps."""
    x = np.asarray(inputs["x"], np.float32)
    rnn = np.asarray(inputs["rnn_input"], np.float32)
    deltas = np.asarray(inputs["deltas"], np.float32)[..., 0]     # [B,T]
    lengths = np.asarray(inputs["lengths"], np.int64)
    idx = np.clip(lengths - 1, 0, T - 1)                          # [B]

    # modified deltas: d for t<idx, d^2 at t==idx, 0 after
    tt = np.arange(T)[None, :]
    dmod = np.where(tt < idx[:, None], deltas,
                    np.where(tt == idx[:, None], deltas * deltas, 0.0)
                    ).astype(np.float32)

    # gate order [i|f|o|g]; within each gate, z order permuted to [h0|x]
    b = (np.asarray(inputs["bih"], np.float32)
         + np.asarray(inputs["bhh"], np.float32))
    perm_z = np.concatenate([np.arange(SD, Z), np.arange(0, SD)])
    gate_perm = np.concatenate([np.arange(0, Z), np.arange(Z, 2 * Z),
                                np.arange(3 * Z, 4 * Z), np.arange(2 * Z, 3 * Z)])
    col_perm = np.concatenate([gate_perm[blk * Z + perm_z] for blk in range(4)])
    wih = np.asarray(inputs["Wih"], np.float32)[:, col_perm]
    whh = np.asarray(inputs["Whh"], np.float32)[np.ix_(perm_z, col_perm)]
    bih_aug = b[col_perm][None, :]                                # [1, 288]
    wih_aug = np.concatenate([wih, bih_aug], axis=0)              # [6, 288]

    ones = np.ones((B, T, 1), np.float32)
    u_aug = np.concatenate([rnn, ones], axis=2)                   # [B, T, 6]

    wd2 = np.asarray(inputs["Wd2"], np.float32)
    wd3 = np.asarray(inputs["Wd3"], np.float32)
    wd1p = np.asarray(inputs["Wd1"], np.float32)[perm_z]
    constsf = {
        "we1": np.asarray(inputs["We1"], np.float32),
        "we2": np.asarray(inputs["We2"], np.float32),
        "we3": np.asarray(inputs["We3"], np.float32),
        "be1": np.asarray(inputs["be1"], np.float32).reshape(ENC_H, 1),
        "be2": np.asarray(inputs["be2"], np.float32).reshape(ENC_H, 1),
        "be3": np.asarray(inputs["be3"], np.float32).reshape(CRS, 1),
        "bd1": np.asarray(inputs["bd1"], np.float32)[0:128].reshape(128, 1),
        "bd1b": np.asarray(inputs["bd1"], np.float32)[128:].reshape(16, 1),
        "bd2": np.asarray(inputs["bd2"], np.float32)[0:128].reshape(128, 1),
        "bd2b": np.asarray(inputs["bd2"], np.float32)[128:].reshape(16, 1),
        "bd3": np.asarray(inputs["bd3"], np.float32).reshape(OUT, 1),
        "ident": np.eye(Z, dtype=np.float32),
    }
    constsb = {
        "wih": wih_aug,
        "whh": whh,
        "wd1": wd1p,
        "wd2a": wd2[0:128],
        "wd2b": wd2[128:DEC_H],
        "wd3a": wd3[0:128],
        "wd3b": wd3[128:DEC_H],
        "identb": np.eye(Z, dtype=np.float32),
    }

    base_packf = np.zeros((128, PACKF_COLS), np.float32)
    for name, arr in constsf.items():
        r, c0, c = _PACKF[name]
        assert arr.shape == (r, c), (name, arr.shape, (r, c))
        base_packf[0:r, c0:c0 + c] = arr
    base_packb = np.zeros((128, PACKB_COLS), ml_dtypes.bfloat16)
    for name, arr in constsb.items():
        r, c0, c = _PACKB[name]
        assert arr.shape == (r, c), (name, arr.shape, (r, c))
        base_packb[0:r, c0:c0 + c] = arr.astype(ml_dtypes.bfloat16)

    in_maps = []
    for k in range(NCORES):
        rows = slice(k * BC, (k + 1) * BC)
        p = base_packf.copy()
        r, c0, c = _PACKF["xfm"]
        p[0:r, c0:c0 + c] = x[rows].T
        r, c0, c = _PACKF["dmod"]
        p[0:r, c0:c0 + c] = dmod[rows]
        r, c0, c = _PACKF["dmod2"]
        p[0:r, c0:c0 + c] = 1.0 - dmod[rows]
        m = {
            "packf": p,
            "packb": base_packb,
            "u": np.ascontiguousarray(u_aug[rows].transpose(2, 1, 0)
                                      ).reshape(6, T * BC
                                                ).astype(ml_dtypes.bfloat16),
        }
        in_maps.append(m)
    return in_maps


def kernel(**inputs):
    nc = _get_nc()
    in_maps = _prep_core_inputs(inputs)
    res = run_bass_kernel_spmd(nc, in_maps, core_ids=list(range(NCORES)))
    outs = [np.asarray(res.results[k]["y"], np.float32).T
            for k in range(NCORES)]                      # each [BC, OUT]
    return np.ascontiguousarray(np.concatenate(outs, axis=0).astype(np.float32))
